# revision 1
# baseline (speedup 1.0000x reference)
"""Two-layer GAT (PyG-style GATConv x2) on 8 Trainium2 NeuronCores.

Design (v2, "host-expand"): nodes are sharded across the 8 cores by
destination. Between launches the HOST rearranges device-computed tables
(pure data movement: fancy-indexed row expansion per edge, sorting,
padding, hi/lo bf16 splits). All model arithmetic (matmuls, logit
add/leaky-relu/exp, softmax division, weighting, ELU, bias) runs on
device.

Rationale: per-edge SWDGE dma_gather costs ~8.3ns/edge of *serial* Q7
descriptor generation (~0.9ms/layer/core) - the measured bottleneck of
the v1 kernel. Pre-expanding edge payload rows on the host turns the
edge pass into dense sequential DMA + one-hot segment-sum matmuls.

Per-edge layout: edges are sorted by dst tile; each dst tile's edges are
padded to a multiple of 128 ("chunks"). Tiles are assigned to "slots" in
decreasing-count order per core so chunk counts align across the 8 SPMD
cores with minimal padding (the host un-permutes outputs).

Layer 1 packs 4 edges of the same dst node into one 260-wide row
(4 x (64 feats + w)), quartering the one-hot matmul count.

Three SPMD launches with host-side expansion between them:
  1. table0: h0^T = W0e^T @ x^T  -> feat-major table + per-node alphas
  2. layer-0 edges: stream payload/softmax/one-hot matmul -> ELU ->
     h1 = h0' @ W1e -> table1 (feat-major) + alphas
  3. layer-1 edges (quad-packed): same -> bias -> output shard
"""

import os

import numpy as np

import concourse.bacc as bacc
import concourse.mybir as mybir
from concourse import tile
from concourse.bass_utils import run_bass_kernel_spmd

fp32 = mybir.dt.float32
bf16 = mybir.dt.bfloat16
Alu = mybir.AluOpType
Act = mybir.ActivationFunctionType

NCORES = 8
NEG_SLOPE = 0.2
EPS = 1e-16
PAD_LOGIT = -30000.0
CPC = 16  # chunks per payload DMA call


def _dims():
    return dict(
        N=50000,
        NLOC=6250,
        NP=6272,  # padded to mult of 128
        NT=49,
        F_IN=256,
        HID=256,
        H=4,
        DH=64,
        C_OUT=64,
    )


# ---------------------------------------------------------------- launch 1


def build_l1(d):
    """h0^T = W0e^T @ x^T per core; W0e = [W0 | W0@A0] folds the per-node
    attention alphas into the same matmul. Outputs feat-major bf16 table
    plus fp32 alphas (host splits hi/lo)."""
    nc = bacc.Bacc(None, target_bir_lowering=False, debug=False)
    NP, F = d["NP"], d["F_IN"]

    xT = nc.dram_tensor("xT", [F, NP], bf16, kind="ExternalInput")
    W0e = nc.dram_tensor("W0e", [F, 264], bf16, kind="ExternalInput")
    t0T = nc.dram_tensor("t0T", [256, NP], bf16, kind="ExternalOutput")
    alT = nc.dram_tensor("alT", [8, NP], fp32, kind="ExternalOutput")

    TW = 512
    n_t = (NP + TW - 1) // TW

    with tile.TileContext(nc) as tc:
        with (
            tc.tile_pool(name="const", bufs=1) as cpool,
            tc.tile_pool(name="work", bufs=3) as pool,
            tc.tile_pool(name="psum", bufs=2, space="PSUM") as pp,
        ):
            w_sb = [
                cpool.tile([128, 264], bf16, tag=f"w{k}", name=f"w{k}")
                for k in range(2)
            ]
            for k in range(2):
                nc.sync.dma_start(w_sb[k][:], W0e[128 * k : 128 * (k + 1), :])

            for t in range(n_t):
                c0 = t * TW
                cw = min(TW, NP - c0)
                xt = [
                    pool.tile([128, TW], bf16, tag=f"xt{k}", name=f"xt{k}")
                    for k in range(2)
                ]
                for k in range(2):
                    nc.sync.dma_start(
                        xt[k][:, :cw], xT[128 * k : 128 * (k + 1), c0 : c0 + cw]
                    )
                for m in range(2):
                    ps = pp.tile([128, TW], fp32, tag=f"ps{m}", name=f"ps{m}")
                    for k in range(2):
                        nc.tensor.matmul(
                            ps[:, :cw],
                            w_sb[k][:, 128 * m : 128 * (m + 1)],
                            xt[k][:, :cw],
                            start=(k == 0),
                            stop=(k == 1),
                        )
                    ob = pool.tile([128, TW], bf16, tag=f"ob{m}", name=f"ob{m}")
                    nc.scalar.activation(ob[:, :cw], ps[:, :cw], Act.Copy)
                    nc.sync.dma_start(
                        t0T[128 * m : 128 * (m + 1), c0 : c0 + cw], ob[:, :cw]
                    )
                pa = pp.tile([8, TW], fp32, tag="pa", name="pa")
                for k in range(2):
                    nc.tensor.matmul(
                        pa[:, :cw],
                        w_sb[k][:, 256:264],
                        xt[k][:, :cw],
                        start=(k == 0),
                        stop=(k == 1),
                    )
                oa = pool.tile([8, TW], fp32, tag="oa", name="oa")
                nc.scalar.activation(oa[:, :cw], pa[:, :cw], Act.Copy)
                nc.sync.dma_start(alT[:, c0 : c0 + cw], oa[:, :cw])
    nc.compile()
    return nc


# ------------------------------------------------------------ edge machinery


def _logits_phase(nc, tc, d, L, NCH, ewb):
    """Batched per-edge softmax numerators: ewb = exp(lrelu(as+ad)) from
    hi/lo bf16 pieces, computed up-front for all chunks."""
    NBLK = 6 if NCH > 600 else 2
    nb = (NCH + NBLK - 1) // NBLK
    with tc.tile_pool(name="logit", bufs=2) as pool:
        for b in range(NBLK):
            b0 = b * nb
            bw = min(nb, NCH - b0)
            if bw <= 0:
                break
            lb = pool.tile([128, nb, 16], bf16, tag="lb", name="lb")
            nc.sync.dma_start(lb[:, :bw, :], L[:, b0 : b0 + bw, :])
            e8 = pool.tile([128, nb, 8], fp32, tag="e8", name="e8")
            nc.vector.tensor_tensor(
                e8[:, :bw, :], lb[:, :bw, 0:8], lb[:, :bw, 8:16], op=Alu.add
            )
            e4 = pool.tile([128, nb, 4], fp32, tag="e4", name="e4")
            nc.vector.tensor_tensor(
                e4[:, :bw, :], e8[:, :bw, 0:4], e8[:, :bw, 4:8], op=Alu.add
            )
            nc.vector.scalar_tensor_tensor(
                e4[:, :bw, :],
                e4[:, :bw, :],
                NEG_SLOPE,
                e4[:, :bw, :],
                op0=Alu.mult,
                op1=Alu.max,
            )
            nc.scalar.activation(ewb[:, b0 : b0 + bw, :], e4[:, :bw, :], Act.Exp)


def _edge_pass(nc, tc, d, P, OHD, Ks, ewb, fin, pp):
    """Stream pre-expanded 260-wide payload rows (4 blocks x (64 feats +
    w-slot)), weight by ewb, one-hot segment-sum into per-tile PSUM."""
    NCH = sum(Ks)

    with tc.tile_pool(name="edge", bufs=3) as pool:
        state = dict(ncalls=0, tiles={})

        def emit_call(call):
            c0 = call * CPC
            nch = min(CPC, NCH - c0)
            G = pool.tile([128, CPC, 264], bf16, tag="G", name="G", bufs=6)
            OH = pool.tile([128, CPC, 128], bf16, tag="OH", name="OH", bufs=6)
            # one-hots are host-built and streamed: a DVE is_equal build
            # measures ~2.2ns/elem (no fast uop + broadcast-port penalty),
            # so DMA is the cheaper engine for them
            nc.sync.dma_start(G[:, :nch, :], P[:, c0 : c0 + nch, :])
            nc.scalar.dma_start(OH[:, :nch, :], OHD[:, c0 : c0 + nch, :])
            g4 = G[:, :nch, :].rearrange("p c (h e) -> p c h e", e=66)
            wb = (
                ewb[:, c0 : c0 + nch, :]
                .unsqueeze(3)
                .broadcast_to([128, nch, 4, 66])
            )
            # payload w-slots are 1.0 from the host, so this multiply also
            # writes the per-block softmax-denominator columns
            nc.vector.tensor_tensor(g4, g4, wb, op=Alu.mult)
            return G, OH

        c = 0
        for s in range(len(Ks)):
            ps = pp.tile([128, 264], fp32, tag="ps", name="ps", bufs=4)
            for k in range(Ks[s]):
                call, cin = c // CPC, c % CPC
                if call >= state["ncalls"]:
                    state["tiles"][call] = emit_call(call)
                    state["ncalls"] = call + 1
                    state["tiles"].pop(call - 5, None)
                G, OH = state["tiles"][call]
                nc.tensor.matmul(
                    ps[:],
                    OH[:, cin, :],
                    G[:, cin, :],
                    start=(k == 0),
                    stop=(k == Ks[s] - 1),
                )
                c += 1
            fin(s, ps)


# ---------------------------------------------------------------- launch 2


def build_l2(d, Ks):
    """Layer-0 edge pass (softmax-div + bias + ELU fused in finalize),
    then table1^T = W1e^T @ h0'^T via a DMA-transpose round trip."""
    nc = bacc.Bacc(None, target_bir_lowering=False, debug=False)
    NP, NT, H = d["NP"], d["NT"], d["H"]
    NCH = sum(Ks)

    P = nc.dram_tensor("P", [128, NCH, 264], bf16, kind="ExternalInput")
    L = nc.dram_tensor("L", [128, NCH, 16], bf16, kind="ExternalInput")
    OHD = nc.dram_tensor("OHD", [128, NCH, 128], bf16, kind="ExternalInput")
    W1e = nc.dram_tensor("W1e", [256, 66], bf16, kind="ExternalInput")
    B0 = nc.dram_tensor("B0", [128, 256], bf16, kind="ExternalInput")
    t1T = nc.dram_tensor("t1T", [64, NP], bf16, kind="ExternalOutput")
    a1T = nc.dram_tensor("a1T", [2, NP], fp32, kind="ExternalOutput")

    with tile.TileContext(nc) as tc:
        with (
            tc.tile_pool(name="const", bufs=1) as cpool,
            tc.tile_pool(name="persist", bufs=1) as ipool,
            tc.tile_pool(name="fin", bufs=3) as fpool,
            tc.tile_pool(name="psum", bufs=1, space="PSUM") as pp,
        ):
            b0_sb = cpool.tile([128, 256], bf16)
            nc.sync.dma_start(b0_sb[:], B0[:])
            ewb = ipool.tile([128, NCH, 4], bf16)
            H0 = ipool.tile([128, NT, 256], bf16)

            _logits_phase(nc, tc, d, L, NCH, ewb)

            def fin0(s, ps):
                sb = fpool.tile([128, 264], fp32, tag="sb", name="sb")
                nc.scalar.activation(sb[:], ps[:], Act.Copy)
                pv = sb[:].rearrange("p (h e) -> p h e", h=H)
                dn = fpool.tile([128, H], fp32, tag="dn", name="dn")
                nc.vector.tensor_scalar_add(dn[:], pv[:, :, 64], EPS)
                rec = fpool.tile([128, H], fp32, tag="rec", name="rec")
                nc.vector.reciprocal(rec[:], dn[:])
                xp = fpool.tile([128, 256], bf16, tag="xp", name="xp")
                for h in range(H):
                    nc.scalar.activation(
                        xp[:, 64 * h : 64 * (h + 1)],
                        pv[:, h, 0:64],
                        Act.Copy,
                        scale=rec[:, h : h + 1],
                    )
                z = fpool.tile([128, 256], bf16, tag="z", name="z")
                nc.vector.tensor_tensor(z[:], xp[:], b0_sb[:], op=Alu.add)
                ex = fpool.tile([128, 256], fp32, tag="ex", name="ex")
                nc.scalar.activation(ex[:], z[:], Act.Exp, bias=1.0)
                m1 = fpool.tile([128, 256], fp32, tag="m1", name="m1")
                nc.vector.tensor_scalar_min(m1[:], ex[:], 1.0)
                nc.vector.scalar_tensor_tensor(
                    H0[:, s, :], z[:], -1.0, m1[:], op0=Alu.max, op1=Alu.add
                )

            _edge_pass(nc, tc, d, P, OHD, Ks, ewb, fin0, pp)

            with (
                tc.tile_pool(name="tb1", bufs=3) as tpool,
                tc.tile_pool(name="dram", bufs=1, space="DRAM") as dpool,
                tc.tile_pool(name="tb1psum", bufs=3, space="PSUM") as pp1,
            ):
                h0d = dpool.tile([NP, 256], bf16)
                nc.sync.dma_start(
                    h0d[:].rearrange("(t p) f -> p t f", p=128), H0[:, :, :]
                )
                h0T = [
                    ipool.tile([128, NP], bf16, tag=f"h0T{k}", name=f"h0T{k}")
                    for k in range(2)
                ]
                for k in range(2):
                    nc.sync.dma_start_transpose(
                        h0T[k][:], h0d[:, 128 * k : 128 * (k + 1)]
                    )
                w1_sb = [
                    cpool.tile([128, 66], bf16, tag=f"w1_{k}", name=f"w1_{k}")
                    for k in range(2)
                ]
                for k in range(2):
                    nc.sync.dma_start(w1_sb[k][:], W1e[128 * k : 128 * (k + 1), :])
                TW = 512
                for j in range((NP + TW - 1) // TW):
                    c0 = j * TW
                    cw = min(TW, NP - c0)
                    pt = pp1.tile([66, TW], fp32, tag="pt", name="pt")
                    for k in range(2):
                        nc.tensor.matmul(
                            pt[:, :cw],
                            w1_sb[k][:],
                            h0T[k][:, c0 : c0 + cw],
                            start=(k == 0),
                            stop=(k == 1),
                        )
                    tb = tpool.tile([64, TW], bf16, tag="tb", name="tb")
                    nc.scalar.activation(tb[:, :cw], pt[0:64, :cw], Act.Copy)
                    nc.sync.dma_start(t1T[:, c0 : c0 + cw], tb[:, :cw])
                    ab = tpool.tile([2, TW], fp32, tag="ab", name="ab")
                    nc.scalar.activation(ab[:, :cw], pt[64:66, :cw], Act.Copy)
                    nc.sync.dma_start(a1T[:, c0 : c0 + cw], ab[:, :cw])
    nc.compile()
    return nc


# ---------------------------------------------------------------- launch 3


def build_l3(d, Ks):
    """Layer-1 edge pass, quad-packed (4 same-dst edges per 260-wide row);
    finalize = sum quads, softmax-div, bias."""
    nc = bacc.Bacc(None, target_bir_lowering=False, debug=False)
    NP, C = d["NP"], d["C_OUT"]
    NCH = sum(Ks)

    P = nc.dram_tensor("P", [128, NCH, 264], bf16, kind="ExternalInput")
    L = nc.dram_tensor("L", [128, NCH, 16], bf16, kind="ExternalInput")
    OHD = nc.dram_tensor("OHD", [128, NCH, 128], bf16, kind="ExternalInput")
    B1 = nc.dram_tensor("B1", [128, C], fp32, kind="ExternalInput")
    out = nc.dram_tensor("out", [NP, C], fp32, kind="ExternalOutput")

    with tile.TileContext(nc) as tc:
        with (
            tc.tile_pool(name="const", bufs=1) as cpool,
            tc.tile_pool(name="persist", bufs=1) as ipool,
            tc.tile_pool(name="fin", bufs=3) as fpool,
            tc.tile_pool(name="psum", bufs=1, space="PSUM") as pp,
        ):
            b1_sb = cpool.tile([128, C], fp32)
            nc.sync.dma_start(b1_sb[:], B1[:])
            ewb = ipool.tile([128, NCH, 4], bf16)

            _logits_phase(nc, tc, d, L, NCH, ewb)

            def fin1(s, ps):
                sb = fpool.tile([128, 264], fp32, tag="sb", name="sb")
                nc.scalar.activation(sb[:], ps[:], Act.Copy)
                sv = sb[:].rearrange("p (q e) -> p q e", q=4)
                a01 = fpool.tile([128, 66], fp32, tag="a01", name="a01")
                nc.vector.tensor_tensor(a01[:], sv[:, 0, :], sv[:, 1, :], op=Alu.add)
                a23 = fpool.tile([128, 66], fp32, tag="a23", name="a23")
                nc.vector.tensor_tensor(a23[:], sv[:, 2, :], sv[:, 3, :], op=Alu.add)
                tot = fpool.tile([128, 66], fp32, tag="tot", name="tot")
                nc.vector.tensor_tensor(tot[:], a01[:], a23[:], op=Alu.add)
                dn = fpool.tile([128, 1], fp32, tag="dnq", name="dnq")
                nc.vector.tensor_scalar_add(dn[:], tot[:, 64:65], EPS)
                rec = fpool.tile([128, 1], fp32, tag="recq", name="recq")
                nc.vector.reciprocal(rec[:], dn[:])
                O = fpool.tile([128, C], fp32, tag="O", name="O")
                nc.vector.scalar_tensor_tensor(
                    O[:], tot[:, 0:64], rec[:], b1_sb[:], op0=Alu.mult, op1=Alu.add
                )
                nc.sync.dma_start(out[128 * s : 128 * (s + 1), :], O[:])

            _edge_pass(nc, tc, d, P, OHD, Ks, ewb, fin1, pp)
    nc.compile()
    return nc


# ------------------------------------------------------------ host plumbing


def _bf16(a):
    import ml_dtypes

    return np.asarray(a).astype(ml_dtypes.bfloat16)


def _hilo(a):
    """fp32 array -> (hi, lo) bf16 with hi+lo ~= a."""
    hi = _bf16(a)
    lo = _bf16(a - hi.astype(np.float32))
    return hi, lo


def _build_A0(att_src, att_dst):
    H, DH = att_src.shape
    A = np.zeros((H * DH, 2 * H), np.float32)
    for h in range(H):
        A[h * DH : (h + 1) * DH, h] = att_src[h]
        A[h * DH : (h + 1) * DH, H + h] = att_dst[h]
    return A


def _prep_edges(edge_index, d):
    """Per-core slot structure for both layers.

    l2 (per-edge): slots = dst tiles sorted by edge count (desc) per core;
    K2[s] = max over cores of ceil(count/128).
    l3 (quad): 4 same-dst edges per row; slots = tiles sorted by quad
    count. Returns per-core index arrays into the node tables.
    """
    N, NLOC, NT = d["N"], d["NLOC"], d["NT"]
    src = np.concatenate([edge_index[0], np.arange(N, dtype=np.int64)])
    dst = np.concatenate([edge_index[1], np.arange(N, dtype=np.int64)])
    core = dst // NLOC

    percore = []
    for c in range(NCORES):
        m = core == c
        s_c, t_c = src[m], dst[m] - c * NLOC
        order = np.argsort(t_c, kind="stable")
        percore.append((s_c[order], t_c[order]))

    # ---- layer-0 structure (per edge)
    counts2 = np.zeros((NCORES, NT), np.int64)
    for c in range(NCORES):
        counts2[c] = np.bincount(percore[c][1] // 128, minlength=NT)
    perm2 = np.argsort(-counts2, axis=1, kind="stable")  # [core, slot] -> tile
    sorted2 = -np.sort(-counts2, axis=1)
    K2 = tuple(int(k) for k in np.ceil(sorted2.max(axis=0) / 128).astype(int))
    NCH2 = sum(K2)
    base2 = np.concatenate([[0], np.cumsum(np.array(K2) * 128)])

    l2 = []
    for c in range(NCORES):
        s_c, t_c = percore[c]
        tile_of = t_c // 128
        EP = NCH2 * 128
        gsrc = np.zeros(EP, np.int64)
        gdst = np.zeros(EP, np.int64)
        rr = np.full(EP, -1.0, np.float32)
        pad = np.ones(EP, bool)
        offs = np.concatenate([[0], np.cumsum(counts2[c][perm2[c]])])
        # edges are tile-sorted; index ranges per tile:
        tstart = np.concatenate([[0], np.cumsum(counts2[c])])
        for s in range(NT):
            tl = perm2[c][s]
            n = counts2[c][tl]
            sl = slice(tstart[tl], tstart[tl] + n)
            b = base2[s]
            gsrc[b : b + n] = s_c[sl]
            gdst[b : b + n] = t_c[sl] + c * NLOC
            rr[b : b + n] = (t_c[sl] - 128 * tl).astype(np.float32)
            pad[b : b + n] = False
        l2.append(dict(gsrc=gsrc, gdst=gdst, rr=rr, pad=pad))

    # ---- layer-1 structure (quads)
    counts3 = np.zeros((NCORES, NT), np.int64)
    quads_pc = []
    for c in range(NCORES):
        s_c, t_c = percore[c]
        deg = np.bincount(t_c, minlength=NLOC)
        nq = (deg + 3) // 4  # quads per node
        counts3[c] = np.add.reduceat(
            nq, np.arange(0, NLOC, 128)
        )
        quads_pc.append((s_c, t_c, deg, nq))
    perm3 = np.argsort(-counts3, axis=1, kind="stable")
    sorted3 = -np.sort(-counts3, axis=1)
    K3 = tuple(int(k) for k in np.ceil(sorted3.max(axis=0) / 128).astype(int))
    NCH3 = sum(K3)
    base3 = np.concatenate([[0], np.cumsum(np.array(K3) * 128)])

    l3 = []
    for c in range(NCORES):
        s_c, t_c, deg, nq = quads_pc[c]
        EP = NCH3 * 128
        qsrc = np.zeros((EP, 4), np.int64)
        qdst = np.zeros(EP, np.int64)
        rr = np.full(EP, -1.0, np.float32)
        pad = np.ones((EP, 4), bool)
        estart = np.concatenate([[0], np.cumsum(deg)])
        qstart_tile = np.concatenate(
            [[0], np.cumsum(counts3[c])]
        )  # quad offset per tile (in tile order)
        for s in range(NT):
            tl = perm3[c][s]
            b = base3[s]
            q = 0
            n0 = tl * 128
            n1 = min(n0 + 128, NLOC)
            for node in range(n0, n1):
                dg = deg[node]
                if dg == 0:
                    continue
                e0 = estart[node]
                nqn = nq[node]
                rows = b + q + np.arange(nqn)
                rr[rows] = float(node - n0)
                qdst[rows] = node + c * NLOC
                es = s_c[e0 : e0 + dg]
                full = np.zeros(nqn * 4, np.int64)
                full[:dg] = es
                qsrc[rows] = full.reshape(nqn, 4)
                pd = np.ones(nqn * 4, bool)
                pd[:dg] = False
                pad[rows] = pd.reshape(nqn, 4)
                q += nqn
        l3.append(dict(qsrc=qsrc, qdst=qdst, rr=rr, pad=pad))

    return dict(K2=K2, K3=K3, perm2=perm2, perm3=perm3, l2=l2, l3=l3)


_EYEP = None


def _oh_rows(rr):
    """rr [EP] (float, -1 = padding) -> one-hot rows [EP, 128] bf16."""
    global _EYEP
    if _EYEP is None:
        _EYEP = np.zeros((129, 128), np.float32)
        _EYEP[:128] = np.eye(128, dtype=np.float32)
        _EYEP = _bf16(_EYEP)
    idx = rr.astype(np.int64)
    idx[idx < 0] = 128
    return _EYEP[idx]


def _pack_pm(a, nch):
    """[EP, W] row-major -> [128, nch, W] partition-major contiguous."""
    W = a.shape[1]
    return np.ascontiguousarray(a.reshape(nch, 128, W).transpose(1, 0, 2))


def _expand_l2(core_idx, tab0, a0, prep):
    """Per-core launch-2 inputs from full node tables (pure gather)."""
    K2 = prep["K2"]
    NCH = sum(K2)
    e = prep["l2"][core_idx]
    gsrc, gdst, pad = e["gsrc"], e["gdst"], e["pad"]
    EP = NCH * 128
    rows = tab0[gsrc]  # [EP, 256] bf16
    P = np.zeros((EP, 264), rows.dtype)
    pv = P.reshape(EP, 4, 66)
    pv[:, :, 0:64] = rows.reshape(EP, 4, 64)
    pv[:, :, 64] = 1.0  # weighting writes w into these denominator slots
    as_hi, as_lo = a0["as_hi"][gsrc], a0["as_lo"][gsrc]
    ad_hi, ad_lo = a0["ad_hi"][gdst], a0["ad_lo"][gdst]
    L = np.concatenate([as_hi, as_lo, ad_hi, ad_lo], axis=1)
    L[pad, 0:4] = PAD_LOGIT
    return dict(
        P=_pack_pm(P, NCH),
        L=_pack_pm(L, NCH),
        OHD=_pack_pm(_oh_rows(e["rr"]), NCH),
    )


def _expand_l3(core_idx, tab1, a1, prep):
    K3 = prep["K3"]
    NCH = sum(K3)
    e = prep["l3"][core_idx]
    qsrc, qdst, pad = e["qsrc"], e["qdst"], e["pad"]
    EP = NCH * 128
    P = np.zeros((EP, 264), tab1.dtype)
    pv = P.reshape(EP, 4, 66)
    for j in range(4):
        pv[:, j, 0:64] = tab1[qsrc[:, j]]
    pv[:, :, 64] = 1.0  # weighting writes w into these denominator slots
    as_hi = a1["as_hi"][qsrc]  # [EP, 4]
    as_lo = a1["as_lo"][qsrc]
    ad_hi = np.repeat(a1["ad_hi"][qdst][:, None], 4, axis=1)
    ad_lo = np.repeat(a1["ad_lo"][qdst][:, None], 4, axis=1)
    L = np.concatenate([as_hi, as_lo, ad_hi, ad_lo], axis=1)
    L[:, 0:4][pad] = PAD_LOGIT
    return dict(
        P=_pack_pm(P, NCH),
        L=_pack_pm(L, NCH),
        OHD=_pack_pm(_oh_rows(e["rr"]), NCH),
    )


_cache = {}
LAST_PROFILE = {}


def _run(nc, in_maps, core_ids, label):
    trace = bool(int(os.environ.get("GAT_PROFILE", "0")))
    if trace:
        try:
            import sys

            import profile_hook

            profile_hook.install()
            import concourse.bass_utils as bu

            bu.upload_artifacts = lambda tmpdir: "local://skipped"
            tdir = f"/tmp/gat_trace_{label}"
            os.makedirs(tdir, exist_ok=True)
            for f in os.listdir(tdir):
                os.unlink(os.path.join(tdir, f))
            br = run_bass_kernel_spmd(nc, in_maps, core_ids, trace=True, tmpdir=tdir)
            LAST_PROFILE[label] = br.exec_time_ns
            return br.results
        except Exception as e:  # fall back to untraced
            print(f"traced run failed ({e!r}); untraced retry", file=sys.stderr)
    br = run_bass_kernel_spmd(nc, in_maps, core_ids)
    LAST_PROFILE[label] = br.exec_time_ns
    return br.results


def kernel(x, edge_index, W0, att_src0, att_dst0, b0, W1, att_src1, att_dst1, b1):
    x = np.asarray(x, np.float32)
    edge_index = np.asarray(edge_index)
    d = _dims()
    N, NLOC, NP, NT = d["N"], d["NLOC"], d["NP"], d["NT"]

    prep = _prep_edges(edge_index, d)
    key = (prep["K2"], prep["K3"])
    if key not in _cache:
        _cache[key] = (build_l1(d), build_l2(d, prep["K2"]), build_l3(d, prep["K3"]))
    nc1, nc2, nc3 = _cache[key]

    A0 = _build_A0(np.asarray(att_src0), np.asarray(att_dst0))
    W0f = np.asarray(W0, np.float32)
    W0e = _bf16(np.concatenate([W0f, W0f @ A0], axis=1))
    W1f = np.asarray(W1, np.float32)
    was1 = W1f @ np.asarray(att_src1, np.float32).ravel()
    wad1 = W1f @ np.asarray(att_dst1, np.float32).ravel()
    W1e = _bf16(np.stack([*W1f.T, was1, wad1], axis=1))  # [256, 66]
    b0m1 = np.tile(np.asarray(b0, np.float32)[None, :] - 1.0, (128, 1))
    b1r = np.tile(np.asarray(b1, np.float32)[None, :], (128, 1))
    core_ids = list(range(NCORES))

    # launch 1
    xb = _bf16(x)
    in1 = []
    for c in range(NCORES):
        xT = np.zeros((d["F_IN"], NP), xb.dtype)
        xT[:, :NLOC] = xb[c * NLOC : (c + 1) * NLOC].T
        in1.append(dict(xT=xT, W0e=W0e))
    r1 = _run(nc1, in1, core_ids, "l1")

    tab0 = np.ascontiguousarray(
        np.concatenate(
            [r1[c]["t0T"][:, :NLOC] for c in range(NCORES)], axis=1
        ).T
    )  # [N, 256] bf16
    alf = np.concatenate([r1[c]["alT"][:, :NLOC] for c in range(NCORES)], axis=1)
    as_hi, as_lo = _hilo(alf[0:4].T)
    ad_hi, ad_lo = _hilo(alf[4:8].T)
    a0 = dict(as_hi=as_hi, as_lo=as_lo, ad_hi=ad_hi, ad_lo=ad_lo)

    in2 = [
        dict(
            _expand_l2(c, tab0, a0, prep),
            W1e=W1e,
            B0=_bf16(b0m1),
        )
        for c in range(NCORES)
    ]
    r2 = _run(nc2, in2, core_ids, "l2")

    # un-permute slot-major table1 columns -> node order
    tab1 = np.zeros((N, 64), r2[0]["t1T"].dtype)
    a1sh = np.zeros(N, np.float32)
    a1dh = np.zeros(N, np.float32)
    for c in range(NCORES):
        t1 = r2[c]["t1T"]  # [64, NP] slot-major
        a1c = r2[c]["a1T"]  # [2, NP]
        for s in range(NT):
            tl = prep["perm2"][c][s]
            n0 = tl * 128
            n1 = min(n0 + 128, NLOC)
            w = n1 - n0
            if w <= 0:
                continue
            tab1[c * NLOC + n0 : c * NLOC + n1] = t1[:, 128 * s : 128 * s + w].T
            a1sh[c * NLOC + n0 : c * NLOC + n1] = a1c[0, 128 * s : 128 * s + w]
            a1dh[c * NLOC + n0 : c * NLOC + n1] = a1c[1, 128 * s : 128 * s + w]
    s_hi, s_lo = _hilo(a1sh)
    d_hi, d_lo = _hilo(a1dh)
    a1 = dict(as_hi=s_hi, as_lo=s_lo, ad_hi=d_hi, ad_lo=d_lo)

    in3 = [
        dict(_expand_l3(c, tab1, a1, prep), B1=b1r)
        for c in range(NCORES)
    ]
    r3 = _run(nc3, in3, core_ids, "l3")

    out = np.zeros((N, 64), np.float32)
    for c in range(NCORES):
        o = r3[c]["out"]  # [NP, 64] slot-major
        for s in range(NT):
            tl = prep["perm3"][c][s]
            n0 = tl * 128
            n1 = min(n0 + 128, NLOC)
            w = n1 - n0
            if w <= 0:
                continue
            out[c * NLOC + n0 : c * NLOC + n1] = o[128 * s : 128 * s + w]
    return out



# revision 7
# speedup vs baseline: 1.2789x; 1.2789x over previous
"""Two-layer GAT (PyG-style GATConv x2) on 8 Trainium2 NeuronCores.

Design (v3, "rank-identity"): nodes are sharded across the 8 cores by
destination and, per core, PERMUTED BY DEGREE (rank order). Edge rows are
laid out so that chunk k of slot s holds the k-th edge of each of the 128
nodes ranked [128s, 128s+128) -- the segment-sum's placement matrix is then
the IDENTITY for every chunk (loaded from SBUF, never streamed from HBM),
and the softmax denominators ride in 4 w-slot columns. The degree sort
makes per-tile chunk counts nearly uniform so padding stays ~2%.

Payload rows use an INTERLEAVED head layout (col = e*4 + h) so the
per-edge attention weighting runs in the DVE's 2x perf mode (stride-1
last dim; measured 2x vs the blocked layout).

All model arithmetic (matmuls, logits, exp, softmax division, weighting,
ELU, bias) runs on device; the host only gathers/permutes/pads rows and
converts dtypes between the three SPMD launches:
  1. t0[rank, :] = [x@W0 (interleaved) | as | ad] per core (node-major)
  2. layer-0 edge pass -> ELU -> fused per-slot transpose + @W1e
     -> t1T = [feats | as1 | ad1] (rank-major columns)
  3. layer-1 edge pass (quad-packed rows, 4 same-dst edges interleaved)
     -> bias -> output shard (rank-major rows)
"""

import os

import numpy as np

import concourse.bacc as bacc
import concourse.mybir as mybir
from concourse import tile
from concourse.bass_utils import run_bass_kernel_spmd

fp32 = mybir.dt.float32
bf16 = mybir.dt.bfloat16
Alu = mybir.AluOpType
Act = mybir.ActivationFunctionType

NCORES = 8
NEG_SLOPE = 0.2
EPS = 1e-16
PAD_LOGIT = -30000.0
CPC = 16  # chunks per payload DMA call
RW = 260  # row width: 65 groups x 4 lanes (64 feat groups + w-slot group)


def _dims():
    return dict(
        N=50000,
        NLOC=6250,
        NP=6272,  # padded to mult of 128
        NT=49,
        F_IN=256,
        HID=256,
        H=4,
        DH=64,
        C_OUT=64,
    )


# ---------------------------------------------------------------- launch 1


def build_l1(d):
    """t0[rank, :] = [x@W0e] node-major per core; W0e = [W0-interleaved |
    W0@A0] folds the per-node attention alphas into the same matmul.
    Stationary = x^T tiles (rank-ordered columns), moving = W0e."""
    nc = bacc.Bacc(None, target_bir_lowering=False, debug=False)
    NP, F, NT = d["NP"], d["F_IN"], d["NT"]

    xT = nc.dram_tensor("xT", [F, NP], bf16, kind="ExternalInput")
    W0e = nc.dram_tensor("W0e", [F, 264], bf16, kind="ExternalInput")
    t0 = nc.dram_tensor("t0", [NP, 264], bf16, kind="ExternalOutput")

    with tile.TileContext(nc) as tc:
        with (
            tc.tile_pool(name="const", bufs=1) as cpool,
            tc.tile_pool(name="work", bufs=3) as pool,
            tc.tile_pool(name="psum", bufs=3, space="PSUM") as pp,
        ):
            w_sb = [
                cpool.tile([128, 264], bf16, tag=f"w{k}", name=f"w{k}")
                for k in range(2)
            ]
            xt = [
                cpool.tile([128, NP], bf16, tag=f"xt{k}", name=f"xt{k}")
                for k in range(2)
            ]
            for k in range(2):
                nc.sync.dma_start(w_sb[k][:], W0e[128 * k : 128 * (k + 1), :])
                nc.scalar.dma_start(xt[k][:], xT[128 * k : 128 * (k + 1), :])

            for j in range(NT):
                c0 = j * 128
                ps = pp.tile([128, 264], fp32, tag="ps", name="ps")
                for k in range(2):
                    nc.tensor.matmul(
                        ps[:],
                        xt[k][:, c0 : c0 + 128],
                        w_sb[k][:],
                        start=(k == 0),
                        stop=(k == 1),
                    )
                ob = pool.tile([128, 264], bf16, tag="ob", name="ob")
                nc.scalar.activation(ob[:], ps[:], Act.Copy)
                nc.sync.dma_start(t0[c0 : c0 + 128, :], ob[:])
    nc.compile()
    return nc


# ------------------------------------------------------------ edge machinery


def _edge_stream(nc, tc, d, P, LAS, AD, Ks, idn, fin, pp):
    """Shared edge pass for both layers.

    Logits: ewb = exp(lrelu(as + ad_slot)), as per edge row, ad per slot.
    Stream: per call, DMA CPC chunks of 260-wide interleaved payload rows,
    weight by ewb (DVE 2x mode), then per chunk accumulate into the slot
    psum via an identity-stationary matmul (placement = row position).
    """
    NT = d["NT"]
    NCH = sum(Ks)
    base = np.concatenate([[0], np.cumsum(Ks)])

    with (
        tc.tile_pool(name="logit", bufs=1) as lpool,
        tc.tile_pool(name="edge", bufs=1) as epool,
    ):
        las = lpool.tile([128, NCH, 4], bf16)
        ewb = lpool.tile([128, NCH, 4], bf16)
        ad = lpool.tile([128, NT, 4], bf16)
        nc.scalar.dma_start(las[:], LAS[:])
        nc.scalar.dma_start(ad[:], AD[:])

        # ---- logits phase: per slot, e = as + ad[s]; lrelu; exp
        for s in range(NT):
            b0, k = base[s], Ks[s]
            if k <= 0:
                continue
            e_s = las[:, b0 : b0 + k, :]
            adb = ad[:, s : s + 1, :].broadcast_to([128, k, 4])
            nc.vector.tensor_tensor(e_s, e_s, adb, op=Alu.add)
            nc.vector.scalar_tensor_tensor(
                e_s, e_s, NEG_SLOPE, e_s, op0=Alu.mult, op1=Alu.max
            )
            nc.scalar.activation(ewb[:, b0 : b0 + k, :], e_s, Act.Exp)

        # ---- edge streaming
        state = dict(ncalls=0, tiles={})

        def emit_call(call):
            c0 = call * CPC
            nch = min(CPC, NCH - c0)
            G = epool.tile([128, CPC, RW], bf16, tag="G", name="G", bufs=6)
            q = nc.sync if call % 2 == 0 else nc.scalar
            q.dma_start(G[:, :nch, :], P[:, c0 : c0 + nch, :])
            g4 = G[:, :nch, :].rearrange("p c (e h) -> p c e h", h=4)
            wb = (
                ewb[:, c0 : c0 + nch, :]
                .unsqueeze(2)
                .broadcast_to([128, nch, RW // 4, 4])
            )
            # payload w-slots are 1.0 from the host, so this multiply also
            # writes the per-lane softmax-denominator columns
            nc.vector.tensor_tensor(g4, g4, wb, op=Alu.mult)
            return G

        c = 0
        for s in range(NT):
            ps = pp.tile([128, RW], fp32, tag="ps", name="ps", bufs=4)
            for k in range(Ks[s]):
                call, cin = c // CPC, c % CPC
                if call >= state["ncalls"]:
                    state["tiles"][call] = emit_call(call)
                    state["ncalls"] = call + 1
                    state["tiles"].pop(call - 5, None)
                G = state["tiles"][call]
                nc.tensor.matmul(
                    ps[:],
                    idn[:],
                    G[:, cin, :],
                    start=(k == 0),
                    stop=(k == Ks[s] - 1),
                )
                c += 1
            fin(s, ps)


# ---------------------------------------------------------------- launch 2


def build_l2(d, Ks):
    """Layer-0 edge pass (softmax-div + bias + ELU in finalize), fused with
    a per-slot PE-transpose + @W1e tail -> t1T columns (rank-major)."""
    nc = bacc.Bacc(None, target_bir_lowering=False, debug=False)
    NP, NT = d["NP"], d["NT"]
    NCH = sum(Ks)

    P = nc.dram_tensor("P", [128, NCH, RW], bf16, kind="ExternalInput")
    LAS = nc.dram_tensor("LAS", [128, NCH, 4], bf16, kind="ExternalInput")
    AD = nc.dram_tensor("AD", [128, NT, 4], bf16, kind="ExternalInput")
    IDN = nc.dram_tensor("IDN", [128, 128], bf16, kind="ExternalInput")
    W1e = nc.dram_tensor("W1e", [256, 66], bf16, kind="ExternalInput")
    B0 = nc.dram_tensor("B0", [128, 256], bf16, kind="ExternalInput")
    t1T = nc.dram_tensor("t1T", [66, NP], bf16, kind="ExternalOutput")

    with tile.TileContext(nc) as tc:
        with (
            tc.tile_pool(name="const", bufs=1) as cpool,
            tc.tile_pool(name="persist", bufs=1) as ipool,
            tc.tile_pool(name="fin", bufs=3) as fpool,
            tc.tile_pool(name="psum", bufs=1, space="PSUM") as pp,
            tc.tile_pool(name="tpsum", bufs=1, space="PSUM") as tp,
        ):
            idn = cpool.tile([128, 128], bf16)
            nc.scalar.dma_start(idn[:], IDN[:])
            b0_sb = cpool.tile([128, 256], bf16)
            nc.scalar.dma_start(b0_sb[:], B0[:])
            w1_sb = [
                cpool.tile([128, 66], bf16, tag=f"w1_{k}", name=f"w1_{k}")
                for k in range(2)
            ]
            for k in range(2):
                nc.scalar.dma_start(w1_sb[k][:], W1e[128 * k : 128 * (k + 1), :])
            H0 = ipool.tile([128, NT, 256], bf16)

            def fin0(s, ps):
                pv = ps[:, 0:256].rearrange("p (e h) -> p e h", h=4)
                dn = fpool.tile([128, 4], fp32, tag="dn", name="dn")
                nc.vector.tensor_scalar_add(dn[:], ps[:, 256:260], EPS)
                rec = fpool.tile([128, 4], fp32, tag="rec", name="rec")
                nc.vector.reciprocal(rec[:], dn[:])
                xp = fpool.tile([128, 256], bf16, tag="xp", name="xp")
                xv = xp[:].rearrange("p (e h) -> p e h", h=4)
                for h in range(4):
                    nc.scalar.activation(
                        xv[:, :, h], pv[:, :, h], Act.Copy,
                        scale=rec[:, h : h + 1],
                    )
                z = fpool.tile([128, 256], bf16, tag="z", name="z")
                nc.vector.tensor_tensor(z[:], xp[:], b0_sb[:], op=Alu.add)
                ex = fpool.tile([128, 256], fp32, tag="ex", name="ex")
                nc.scalar.activation(ex[:], z[:], Act.Exp, bias=1.0)
                m1 = fpool.tile([128, 256], fp32, tag="m1", name="m1")
                nc.vector.tensor_scalar_min(m1[:], ex[:], 1.0)
                nc.vector.scalar_tensor_tensor(
                    H0[:, s, :], z[:], -1.0, m1[:], op0=Alu.max, op1=Alu.add
                )
                # fused tail: h0'[slot]^T via PE transpose, then @W1e
                p66 = tp.tile([66, 128], fp32, tag="p66", name="p66", bufs=2)
                for kb in range(2):
                    pt = tp.tile([128, 128], bf16, tag="pt", name="pt", bufs=2)
                    nc.tensor.transpose(
                        pt[:], H0[:, s, 128 * kb : 128 * (kb + 1)], idn[:]
                    )
                    hT = fpool.tile([128, 128], bf16, tag="hT", name="hT")
                    nc.scalar.activation(hT[:], pt[:], Act.Copy)
                    nc.tensor.matmul(
                        p66[:],
                        w1_sb[kb][:],
                        hT[:],
                        start=(kb == 0),
                        stop=(kb == 1),
                    )
                t1b = fpool.tile([66, 128], bf16, tag="t1b", name="t1b")
                nc.scalar.activation(t1b[:], p66[:], Act.Copy)
                nc.sync.dma_start(t1T[:, 128 * s : 128 * (s + 1)], t1b[:])

            _edge_stream(nc, tc, d, P, LAS, AD, Ks, idn, fin0, pp)
    nc.compile()
    return nc


# ---------------------------------------------------------------- launch 3


def build_l3(d, Ks):
    """Layer-1 edge pass, quad-packed (4 same-dst edges per row, lane-
    interleaved); finalize = sum lanes, softmax-div, bias."""
    nc = bacc.Bacc(None, target_bir_lowering=False, debug=False)
    NP, NT, C = d["NP"], d["NT"], d["C_OUT"]
    NCH = sum(Ks)

    P = nc.dram_tensor("P", [128, NCH, RW], bf16, kind="ExternalInput")
    LAS = nc.dram_tensor("LAS", [128, NCH, 4], bf16, kind="ExternalInput")
    AD = nc.dram_tensor("AD", [128, NT, 4], bf16, kind="ExternalInput")
    IDN = nc.dram_tensor("IDN", [128, 128], bf16, kind="ExternalInput")
    B1 = nc.dram_tensor("B1", [128, C], fp32, kind="ExternalInput")
    out = nc.dram_tensor("out", [NP, C], fp32, kind="ExternalOutput")

    with tile.TileContext(nc) as tc:
        with (
            tc.tile_pool(name="const", bufs=1) as cpool,
            tc.tile_pool(name="fin", bufs=3) as fpool,
            tc.tile_pool(name="psum", bufs=1, space="PSUM") as pp,
        ):
            idn = cpool.tile([128, 128], bf16)
            nc.scalar.dma_start(idn[:], IDN[:])
            b1_sb = cpool.tile([128, C], fp32)
            nc.scalar.dma_start(b1_sb[:], B1[:])

            def fin1(s, ps):
                sb = fpool.tile([128, RW], fp32, tag="sb", name="sb")
                nc.scalar.activation(sb[:], ps[:], Act.Copy)
                sv = sb[:].rearrange("p (e q) -> p e q", q=4)
                t2 = fpool.tile([128, 65, 2], fp32, tag="t2", name="t2")
                nc.vector.tensor_tensor(
                    t2[:], sv[:, :, 0:2], sv[:, :, 2:4], op=Alu.add
                )
                tot = fpool.tile([128, 65], fp32, tag="tot", name="tot")
                nc.vector.tensor_tensor(
                    tot[:], t2[:, :, 0], t2[:, :, 1], op=Alu.add
                )
                dn = fpool.tile([128, 1], fp32, tag="dnq", name="dnq")
                nc.vector.tensor_scalar_add(dn[:], tot[:, 64:65], EPS)
                rec = fpool.tile([128, 1], fp32, tag="recq", name="recq")
                nc.vector.reciprocal(rec[:], dn[:])
                O = fpool.tile([128, C], fp32, tag="O", name="O")
                nc.vector.scalar_tensor_tensor(
                    O[:], tot[:, 0:64], rec[:], b1_sb[:], op0=Alu.mult, op1=Alu.add
                )
                nc.sync.dma_start(out[128 * s : 128 * (s + 1), :], O[:])

            _edge_stream(nc, tc, d, P, LAS, AD, Ks, idn, fin1, pp)
    nc.compile()
    return nc


# ------------------------------------------------------------ host plumbing


def _bf16(a):
    import ml_dtypes

    return np.asarray(a).astype(ml_dtypes.bfloat16)


def _prep_edges(edge_index, d):
    """Rank permutations + identity-placement row indices for both layers."""
    N, NLOC, NP, NT = d["N"], d["NLOC"], d["NP"], d["NT"]
    src = np.concatenate([edge_index[0], np.arange(N, dtype=np.int64)])
    dst = np.concatenate([edge_index[1], np.arange(N, dtype=np.int64)])
    core = dst // NLOC

    orders, ranks = [], []
    deg_t, nq_t = [], []
    percore = []
    for c in range(NCORES):
        m = core == c
        s_c, t_c = src[m], (dst[m] - c * NLOC).astype(np.int64)
        percore.append((s_c, t_c))
        deg = np.bincount(t_c, minlength=NLOC)
        order = np.argsort(-deg, kind="stable")
        rank = np.empty(NLOC, np.int64)
        rank[order] = np.arange(NLOC)
        orders.append(order)
        ranks.append(rank)
        dp = np.zeros(NP, np.int64)
        dp[:NLOC] = deg[order]
        deg_t.append(dp.reshape(NT, 128).max(axis=1))
        nqp = np.zeros(NP, np.int64)
        nqp[:NLOC] = (deg[order] + 3) // 4
        nq_t.append(nqp.reshape(NT, 128).max(axis=1))

    K2 = tuple(int(v) for v in np.max(deg_t, axis=0))
    K3 = tuple(int(v) for v in np.max(nq_t, axis=0))
    base2 = np.concatenate([[0], np.cumsum(K2)])
    base3 = np.concatenate([[0], np.cumsum(K3)])
    NCH2, NCH3 = int(base2[-1]), int(base3[-1])

    grow = np.empty(N, np.int64)  # node id -> global table row
    for c in range(NCORES):
        grow[c * NLOC : (c + 1) * NLOC] = c * NP + ranks[c]

    l2rows, l3rows = [], []
    for c in range(NCORES):
        s_c, t_c = percore[c]
        r_t = ranks[c][t_c]
        o = np.argsort(r_t, kind="stable")
        s_e, r_e = grow[s_c[o]], r_t[o]
        first = np.searchsorted(r_e, r_e, side="left")
        k = np.arange(len(r_e)) - first
        slot, row = r_e // 128, r_e % 128
        ridx2 = (base2[slot] + k) * 128 + row
        g2 = np.full(NCH2 * 128, -1, np.int64)
        g2[ridx2] = s_e
        ridx3 = ((base3[slot] + k // 4) * 128 + row) * 4 + (k % 4)
        g3 = np.full(NCH3 * 128 * 4, -1, np.int64)
        g3[ridx3] = s_e
        l2rows.append(g2)
        l3rows.append(g3)

    return dict(
        K2=K2, K3=K3, NCH2=NCH2, NCH3=NCH3,
        orders=orders, l2rows=l2rows, l3rows=l3rows,
    )


def _pack_pm(a, nch):
    """[NCH*128, W] row-major -> [128, nch, W] partition-major contiguous."""
    W = a.shape[1]
    return np.ascontiguousarray(a.reshape(nch, 128, W).transpose(1, 0, 2))


def _expand_l2(c, t0_all, prep):
    NCH = prep["NCH2"]
    g2 = prep["l2rows"][c]
    pad = g2 < 0
    R = t0_all[np.where(pad, 0, g2)]  # [EP, 264] bf16
    P = np.empty((NCH * 128, RW), t0_all.dtype)
    P[:, 0:256] = R[:, 0:256]
    P[:, 256:260] = 1.0  # w-slots: weighting writes denominator terms here
    L = R[:, 256:260].copy()
    L[pad] = PAD_LOGIT
    return dict(P=_pack_pm(P, NCH), LAS=_pack_pm(L, NCH))


def _expand_l3(c, t1_all, prep):
    NCH = prep["NCH3"]
    g3 = prep["l3rows"][c]
    pad = (g3 < 0).reshape(-1, 4)
    R = t1_all[np.where(g3 < 0, 0, g3)]  # [EP*4, 66] bf16
    EP = NCH * 128
    P = np.empty((EP, RW), t1_all.dtype)
    pv = P.reshape(EP, 65, 4)
    pv[:, 0:64, :] = R[:, 0:64].reshape(EP, 4, 64).transpose(0, 2, 1)
    pv[:, 64, :] = 1.0
    L = R[:, 64].reshape(EP, 4).copy()
    L[pad] = PAD_LOGIT
    return dict(P=_pack_pm(P, NCH), LAS=_pack_pm(L, NCH))


_cache = {}
LAST_PROFILE = {}


def _run(nc, in_maps, core_ids, label):
    trace = bool(int(os.environ.get("GAT_PROFILE", "0")))
    if trace:
        try:
            import sys

            import profile_hook

            profile_hook.install()
            import concourse.bass_utils as bu

            bu.upload_artifacts = lambda tmpdir: "local://skipped"
            tdir = f"/tmp/gat_trace_{label}"
            os.makedirs(tdir, exist_ok=True)
            for f in os.listdir(tdir):
                os.unlink(os.path.join(tdir, f))
            br = run_bass_kernel_spmd(nc, in_maps, core_ids, trace=True, tmpdir=tdir)
            LAST_PROFILE[label] = br.exec_time_ns
            return br.results
        except Exception as e:  # fall back to untraced
            print(f"traced run failed ({e!r}); untraced retry", file=sys.stderr)
    br = run_bass_kernel_spmd(nc, in_maps, core_ids)
    LAST_PROFILE[label] = br.exec_time_ns
    return br.results


def kernel(x, edge_index, W0, att_src0, att_dst0, b0, W1, att_src1, att_dst1, b1):
    x = np.asarray(x, np.float32)
    edge_index = np.asarray(edge_index)
    d = _dims()
    N, NLOC, NP, NT = d["N"], d["NLOC"], d["NP"], d["NT"]

    prep = _prep_edges(edge_index, d)
    key = (prep["K2"], prep["K3"])
    if key not in _cache:
        _cache[key] = (build_l1(d), build_l2(d, prep["K2"]), build_l3(d, prep["K3"]))
    nc1, nc2, nc3 = _cache[key]

    # interleave W0 columns: col e*4+h <- W0 col h*64+e; alphas cols 256..263
    W0f = np.asarray(W0, np.float32)
    W0i = np.ascontiguousarray(
        W0f.reshape(256, 4, 64).transpose(0, 2, 1).reshape(256, 256)
    )
    as0 = W0f.reshape(256, 4, 64) * np.asarray(att_src0, np.float32)[None, :, :]
    ad0 = W0f.reshape(256, 4, 64) * np.asarray(att_dst0, np.float32)[None, :, :]
    W0e = _bf16(
        np.concatenate([W0i, as0.sum(axis=2), ad0.sum(axis=2)], axis=1)
    )  # [256, 264]

    # W1e rows permuted to the interleaved feature order
    W1f = np.asarray(W1, np.float32)
    was1 = W1f @ np.asarray(att_src1, np.float32).ravel()
    wad1 = W1f @ np.asarray(att_dst1, np.float32).ravel()
    W1e = np.concatenate([W1f, was1[:, None], wad1[:, None]], axis=1)  # [256, 66]
    perm = (np.arange(256) % 4) * 64 + np.arange(256) // 4
    W1e = _bf16(W1e[perm])

    b0i = np.asarray(b0, np.float32)[(np.arange(256) % 4) * 64 + np.arange(256) // 4]
    B0 = _bf16(np.tile(b0i[None, :] - 1.0, (128, 1)))
    B1 = np.tile(np.asarray(b1, np.float32)[None, :], (128, 1))
    IDN = _bf16(np.eye(128, dtype=np.float32))
    core_ids = list(range(NCORES))

    # launch 1
    xb = _bf16(x)
    in1 = []
    for c in range(NCORES):
        xT = np.zeros((d["F_IN"], NP), xb.dtype)
        xT[:, :NLOC] = xb[c * NLOC : (c + 1) * NLOC][prep["orders"][c]].T
        in1.append(dict(xT=xT, W0e=W0e))
    r1 = _run(nc1, in1, core_ids, "l1")

    t0_all = np.concatenate([r1[c]["t0"] for c in range(NCORES)], axis=0)

    in2 = []
    for c in range(NCORES):
        e = _expand_l2(c, t0_all, prep)
        t0c = t0_all[c * NP : (c + 1) * NP]
        AD = np.ascontiguousarray(
            t0c[:, 260:264].reshape(NT, 128, 4).transpose(1, 0, 2)
        )
        in2.append(dict(e, AD=AD, IDN=IDN, W1e=W1e, B0=B0))
    r2 = _run(nc2, in2, core_ids, "l2")

    t1_all = np.concatenate(
        [np.ascontiguousarray(r2[c]["t1T"].T) for c in range(NCORES)], axis=0
    )  # [8*NP, 66] bf16

    in3 = []
    for c in range(NCORES):
        e = _expand_l3(c, t1_all, prep)
        t1c = t1_all[c * NP : (c + 1) * NP]
        AD = np.ascontiguousarray(
            np.repeat(
                t1c[:, 65].reshape(NT, 128).transpose(1, 0)[:, :, None], 4, axis=2
            )
        )
        in3.append(dict(e, AD=AD, IDN=IDN, B1=B1))
    r3 = _run(nc3, in3, core_ids, "l3")

    out = np.zeros((N, 64), np.float32)
    for c in range(NCORES):
        out[c * NLOC + prep["orders"][c]] = r3[c]["out"][:NLOC]
    return out


# revision 17
# speedup vs baseline: 1.3101x; 1.0244x over previous
"""Two-layer GAT (PyG-style GATConv x2) on 8 Trainium2 NeuronCores.

Design (v3, "rank-identity"): nodes are sharded across the 8 cores by
destination and, per core, PERMUTED BY DEGREE (rank order). Edge rows are
laid out so that chunk k of slot s holds the k-th edge of each of the 128
nodes ranked [128s, 128s+128) -- the segment-sum's placement matrix is then
the IDENTITY for every chunk (loaded from SBUF, never streamed from HBM),
and the softmax denominators ride in 4 w-slot columns. The degree sort
makes per-tile chunk counts nearly uniform so padding stays ~2%.

Payload rows use an INTERLEAVED head layout (col = e*4 + h) so the
per-edge attention weighting runs in the DVE's 2x perf mode (stride-1
last dim; measured 2x vs the blocked layout).

All model arithmetic (matmuls, logits, exp, softmax division, weighting,
ELU, bias) runs on device; the host only gathers/permutes/pads rows and
converts dtypes between the three SPMD launches:
  1. t0[rank, :] = [x@W0 (interleaved) | as | ad] per core (node-major)
  2. layer-0 edge pass -> ELU -> fused per-slot transpose + @W1e
     -> t1T = [feats | as1 | ad1] (rank-major columns)
  3. layer-1 edge pass (quad-packed rows, 4 same-dst edges interleaved)
     -> bias -> output shard (rank-major rows)
"""

import os

import numpy as np

import concourse.bacc as bacc
import concourse.mybir as mybir
from concourse import tile
from concourse.bass_utils import run_bass_kernel_spmd

fp32 = mybir.dt.float32
f16 = mybir.dt.float16
Alu = mybir.AluOpType
Act = mybir.ActivationFunctionType

NCORES = 8
NEG_SLOPE = 0.2
EPS = 1e-16
PAD_LOGIT = -30000.0
CPC = 16  # chunks per payload DMA call
RW = 260  # row width: 65 groups x 4 lanes (64 feat groups + w-slot group)


def _dims():
    return dict(
        N=50000,
        NLOC=6250,
        NP=6272,  # padded to mult of 128
        NT=49,
        F_IN=256,
        HID=256,
        H=4,
        DH=64,
        C_OUT=64,
    )


# ---------------------------------------------------------------- launch 1


def build_l1(d):
    """t0[rank, :] = [x@W0e] node-major per core; W0e = [W0-interleaved |
    W0@A0] folds the per-node attention alphas into the same matmul.
    Stationary = x^T tiles (rank-ordered columns), moving = W0e."""
    nc = bacc.Bacc(None, target_bir_lowering=False, debug=False)
    NP, F, NT = d["NP"], d["F_IN"], d["NT"]

    xT = nc.dram_tensor("xT", [F, NP], f16, kind="ExternalInput")
    W0e = nc.dram_tensor("W0e", [F, 264], f16, kind="ExternalInput")
    t0 = nc.dram_tensor("t0", [NP, 264], f16, kind="ExternalOutput")

    with tile.TileContext(nc) as tc:
        with (
            tc.tile_pool(name="const", bufs=1) as cpool,
            tc.tile_pool(name="work", bufs=3) as pool,
            tc.tile_pool(name="psum", bufs=3, space="PSUM") as pp,
        ):
            w_sb = [
                cpool.tile([128, 264], f16, tag=f"w{k}", name=f"w{k}")
                for k in range(2)
            ]
            xt = [
                cpool.tile([128, NP], f16, tag=f"xt{k}", name=f"xt{k}")
                for k in range(2)
            ]
            for k in range(2):
                nc.sync.dma_start(w_sb[k][:], W0e[128 * k : 128 * (k + 1), :])
                nc.scalar.dma_start(xt[k][:], xT[128 * k : 128 * (k + 1), :])

            TB = 4  # tiles batched per output DMA
            for j0 in range(0, NT, TB):
                nb = min(TB, NT - j0)
                ob = pool.tile([128, TB, 264], f16, tag="ob", name="ob")
                for t in range(nb):
                    j = j0 + t
                    c0 = j * 128
                    ps = pp.tile([128, 264], fp32, tag="ps", name="ps", bufs=4)
                    for k in range(2):
                        nc.tensor.matmul(
                            ps[:],
                            xt[k][:, c0 : c0 + 128],
                            w_sb[k][:],
                            start=(k == 0),
                            stop=(k == 1),
                        )
                    if t % 2 == 0:
                        nc.scalar.activation(ob[:, t, :], ps[:], Act.Copy)
                    else:
                        nc.vector.tensor_copy(ob[:, t, :], ps[:])
                dv = t0[j0 * 128 : (j0 + nb) * 128, :].rearrange(
                    "(t p) f -> p t f", p=128
                )
                nc.sync.dma_start(dv, ob[:, :nb, :])
    nc.compile()
    return nc


# ------------------------------------------------------------ edge machinery


def _edge_stream(nc, tc, d, P, LAS, AD, Ks, idn, fin, pp):
    """Shared edge pass for both layers.

    Logits: ewb = exp(lrelu(as + ad_slot)), as per edge row, ad per slot.
    Stream: per call, DMA CPC chunks of 260-wide interleaved payload rows,
    weight by ewb (DVE 2x mode), then per chunk accumulate into the slot
    psum via an identity-stationary matmul (placement = row position).
    """
    NT = d["NT"]
    NCH = sum(Ks)
    base = np.concatenate([[0], np.cumsum(Ks)])

    with (
        tc.tile_pool(name="logit", bufs=1) as lpool,
        tc.tile_pool(name="edge", bufs=1) as epool,
    ):
        las = lpool.tile([128, NCH, 4], f16)
        ewb = lpool.tile([128, NCH, 4], f16)
        ad = lpool.tile([128, NT, 4], f16)
        nc.scalar.dma_start(las[:], LAS[:])
        nc.scalar.dma_start(ad[:], AD[:])

        # ---- logits phase: per slot, e = as + ad[s]; lrelu; exp
        for s in range(NT):
            b0, k = base[s], Ks[s]
            if k <= 0:
                continue
            e_s = las[:, b0 : b0 + k, :]
            adb = ad[:, s : s + 1, :].broadcast_to([128, k, 4])
            nc.gpsimd.tensor_tensor(e_s, e_s, adb, op=Alu.add)
            nc.vector.scalar_tensor_tensor(
                e_s, e_s, NEG_SLOPE, e_s, op0=Alu.mult, op1=Alu.max
            )
            nc.scalar.activation(ewb[:, b0 : b0 + k, :], e_s, Act.Exp)

        # ---- edge streaming
        state = dict(ncalls=0, tiles={})

        def emit_call(call):
            c0 = call * CPC
            nch = min(CPC, NCH - c0)
            G = epool.tile([128, CPC, RW], f16, tag="G", name="G", bufs=6)
            q = nc.sync if call % 2 == 0 else nc.scalar
            q.dma_start(G[:, :nch, :], P[:, c0 : c0 + nch, :])
            g4 = G[:, :nch, :].rearrange("p c (e h) -> p c e h", h=4)
            wb = (
                ewb[:, c0 : c0 + nch, :]
                .unsqueeze(2)
                .broadcast_to([128, nch, RW // 4, 4])
            )
            # payload w-slots are 1.0 from the host, so this multiply also
            # writes the per-lane softmax-denominator columns
            nc.vector.tensor_tensor(g4, g4, wb, op=Alu.mult)
            return G

        c = 0
        for s in range(NT):
            ps = pp.tile([128, RW], fp32, tag="ps", name="ps", bufs=4)
            for k in range(Ks[s]):
                call, cin = c // CPC, c % CPC
                if call >= state["ncalls"]:
                    state["tiles"][call] = emit_call(call)
                    state["ncalls"] = call + 1
                    state["tiles"].pop(call - 5, None)
                G = state["tiles"][call]
                nc.tensor.matmul(
                    ps[:],
                    idn[:],
                    G[:, cin, :],
                    start=(k == 0),
                    stop=(k == Ks[s] - 1),
                )
                c += 1
            fin(s, ps)


# ---------------------------------------------------------------- launch 2


def build_l2(d, Ks):
    """Layer-0 edge pass (softmax-div + bias + ELU in finalize), fused with
    a per-slot PE-transpose + @W1e tail -> t1T columns (rank-major)."""
    nc = bacc.Bacc(None, target_bir_lowering=False, debug=False)
    NP, NT = d["NP"], d["NT"]
    NCH = sum(Ks)

    P = nc.dram_tensor("P", [128, NCH, RW], f16, kind="ExternalInput")
    LAS = nc.dram_tensor("LAS", [128, NCH, 4], f16, kind="ExternalInput")
    AD = nc.dram_tensor("AD", [128, NT, 4], f16, kind="ExternalInput")
    IDN = nc.dram_tensor("IDN", [128, 128], f16, kind="ExternalInput")
    W1e = nc.dram_tensor("W1e", [256, 66], f16, kind="ExternalInput")
    B0 = nc.dram_tensor("B0", [128, 256], f16, kind="ExternalInput")
    B66 = nc.dram_tensor("B66", [66, 1], fp32, kind="ExternalInput")
    t1T = nc.dram_tensor("t1T", [66, NP], f16, kind="ExternalOutput")

    with tile.TileContext(nc) as tc:
        with (
            tc.tile_pool(name="const", bufs=1) as cpool,
            tc.tile_pool(name="persist", bufs=1) as ipool,
            tc.tile_pool(name="fin", bufs=3) as fpool,
            tc.tile_pool(name="psum", bufs=1, space="PSUM") as pp,
            tc.tile_pool(name="tpsum", bufs=1, space="PSUM") as tp,
        ):
            idn = cpool.tile([128, 128], f16)
            nc.scalar.dma_start(idn[:], IDN[:])
            b0_sb = cpool.tile([128, 256], f16)
            nc.scalar.dma_start(b0_sb[:], B0[:])
            b66_sb = cpool.tile([66, 1], fp32)
            nc.scalar.dma_start(b66_sb[:], B66[:])
            w1_sb = [
                cpool.tile([128, 66], f16, tag=f"w1_{k}", name=f"w1_{k}")
                for k in range(2)
            ]
            for k in range(2):
                nc.scalar.dma_start(w1_sb[k][:], W1e[128 * k : 128 * (k + 1), :])
            H0 = ipool.tile([128, NT, 256], f16)

            def fin0(s, ps):
                # denominators >= exp(-1) for real rows; pad rows produce
                # Inf/NaN and are discarded by the host.
                # ELU via relu/exp only: H0 = relu(u) + exp(-relu(-u))
                # = elu(u) + 1; the +1 shift is corrected by the B66 bias
                # on the t1 copy (t1 is linear in H0).
                pv = ps[:, 0:256].rearrange("p (e h) -> p e h", h=4)
                rec = fpool.tile([128, 4], fp32, tag="rec", name="rec")
                nc.vector.reciprocal(rec[:], ps[:, 256:260])
                xp = fpool.tile([128, 256], f16, tag="xp", name="xp")
                xv = xp[:].rearrange("p (e h) -> p e h", h=4)
                rb = rec[:].unsqueeze(1).broadcast_to([128, 64, 4])
                nc.vector.tensor_tensor(xv, pv, rb, op=Alu.mult)
                z = fpool.tile([128, 256], f16, tag="z", name="z")
                nc.gpsimd.tensor_tensor(z[:], xp[:], b0_sb[:], op=Alu.add)
                ra = fpool.tile([128, 256], f16, tag="ra", name="ra")
                nc.scalar.activation(ra[:], z[:], Act.Relu)
                rn = fpool.tile([128, 256], f16, tag="rn", name="rn")
                nc.scalar.activation(rn[:], z[:], Act.Relu, scale=-1.0)
                ce = fpool.tile([128, 256], f16, tag="ce", name="ce")
                nc.scalar.activation(ce[:], rn[:], Act.Exp, scale=-1.0)
                nc.gpsimd.tensor_tensor(H0[:, s, :], ra[:], ce[:], op=Alu.add)
                # fused tail: h0'[slot]^T via PE transpose, then @W1e
                p66 = tp.tile([66, 128], fp32, tag="p66", name="p66", bufs=2)
                for kb in range(2):
                    pt = tp.tile([128, 128], f16, tag="pt", name="pt", bufs=2)
                    nc.tensor.transpose(
                        pt[:], H0[:, s, 128 * kb : 128 * (kb + 1)], idn[:]
                    )
                    hT = fpool.tile([128, 128], f16, tag="hT", name="hT")
                    if kb == 0:
                        nc.scalar.activation(hT[:], pt[:], Act.Copy)
                    else:
                        nc.vector.tensor_copy(hT[:], pt[:])
                    nc.tensor.matmul(
                        p66[:],
                        w1_sb[kb][:],
                        hT[:],
                        start=(kb == 0),
                        stop=(kb == 1),
                    )
                t1b = fpool.tile([66, 128], f16, tag="t1b", name="t1b")
                nc.scalar.activation(t1b[:], p66[:], Act.Identity, bias=b66_sb[:], scale=1.0)
                nc.sync.dma_start(t1T[:, 128 * s : 128 * (s + 1)], t1b[:])

            _edge_stream(nc, tc, d, P, LAS, AD, Ks, idn, fin0, pp)
    nc.compile()
    return nc


# ---------------------------------------------------------------- launch 3


def build_l3(d, Ks):
    """Layer-1 edge pass, quad-packed (4 same-dst edges per row, lane-
    interleaved); finalize = sum lanes, softmax-div, bias."""
    nc = bacc.Bacc(None, target_bir_lowering=False, debug=False)
    NP, NT, C = d["NP"], d["NT"], d["C_OUT"]
    NCH = sum(Ks)

    P = nc.dram_tensor("P", [128, NCH, RW], f16, kind="ExternalInput")
    LAS = nc.dram_tensor("LAS", [128, NCH, 4], f16, kind="ExternalInput")
    AD = nc.dram_tensor("AD", [128, NT, 4], f16, kind="ExternalInput")
    IDN = nc.dram_tensor("IDN", [128, 128], f16, kind="ExternalInput")
    B1 = nc.dram_tensor("B1", [128, C], fp32, kind="ExternalInput")
    out = nc.dram_tensor("out", [NP, C], fp32, kind="ExternalOutput")

    with tile.TileContext(nc) as tc:
        with (
            tc.tile_pool(name="const", bufs=1) as cpool,
            tc.tile_pool(name="fin", bufs=3) as fpool,
            tc.tile_pool(name="psum", bufs=1, space="PSUM") as pp,
        ):
            idn = cpool.tile([128, 128], f16)
            nc.scalar.dma_start(idn[:], IDN[:])
            b1_sb = cpool.tile([128, C], fp32)
            nc.scalar.dma_start(b1_sb[:], B1[:])

            def fin1(s, ps):
                sb = fpool.tile([128, RW], fp32, tag="sb", name="sb")
                nc.scalar.activation(sb[:], ps[:], Act.Copy)
                sv = sb[:].rearrange("p (e q) -> p e q", q=4)
                t2 = fpool.tile([128, 65, 2], fp32, tag="t2", name="t2")
                nc.gpsimd.tensor_tensor(
                    t2[:], sv[:, :, 0:2], sv[:, :, 2:4], op=Alu.add
                )
                tot = fpool.tile([128, 65], fp32, tag="tot", name="tot")
                nc.gpsimd.tensor_tensor(
                    tot[:], t2[:, :, 0], t2[:, :, 1], op=Alu.add
                )
                rec = fpool.tile([128, 1], fp32, tag="recq", name="recq")
                nc.vector.reciprocal(rec[:], tot[:, 64:65])
                O = fpool.tile([128, C], fp32, tag="O", name="O")
                nc.vector.scalar_tensor_tensor(
                    O[:], tot[:, 0:64], rec[:], b1_sb[:], op0=Alu.mult, op1=Alu.add
                )
                nc.sync.dma_start(out[128 * s : 128 * (s + 1), :], O[:])

            _edge_stream(nc, tc, d, P, LAS, AD, Ks, idn, fin1, pp)
    nc.compile()
    return nc


# ------------------------------------------------------------ host plumbing


def _f16(a):
    return np.asarray(a).astype(np.float16)


def _prep_edges(edge_index, d):
    """Rank permutations + identity-placement row indices for both layers."""
    N, NLOC, NP, NT = d["N"], d["NLOC"], d["NP"], d["NT"]
    src = np.concatenate([edge_index[0], np.arange(N, dtype=np.int64)])
    dst = np.concatenate([edge_index[1], np.arange(N, dtype=np.int64)])
    core = dst // NLOC

    orders, ranks = [], []
    deg_t, nq_t = [], []
    percore = []
    for c in range(NCORES):
        m = core == c
        s_c, t_c = src[m], (dst[m] - c * NLOC).astype(np.int64)
        percore.append((s_c, t_c))
        deg = np.bincount(t_c, minlength=NLOC)
        order = np.argsort(-deg, kind="stable")
        rank = np.empty(NLOC, np.int64)
        rank[order] = np.arange(NLOC)
        orders.append(order)
        ranks.append(rank)
        dp = np.zeros(NP, np.int64)
        dp[:NLOC] = deg[order]
        deg_t.append(dp.reshape(NT, 128).max(axis=1))
        nqp = np.zeros(NP, np.int64)
        nqp[:NLOC] = (deg[order] + 3) // 4
        nq_t.append(nqp.reshape(NT, 128).max(axis=1))

    K2 = tuple(int(v) for v in np.max(deg_t, axis=0))
    K3 = tuple(int(v) for v in np.max(nq_t, axis=0))
    base2 = np.concatenate([[0], np.cumsum(K2)])
    base3 = np.concatenate([[0], np.cumsum(K3)])
    NCH2, NCH3 = int(base2[-1]), int(base3[-1])

    grow = np.empty(N, np.int64)  # node id -> global table row
    for c in range(NCORES):
        grow[c * NLOC : (c + 1) * NLOC] = c * NP + ranks[c]

    l2rows, l3rows = [], []
    for c in range(NCORES):
        s_c, t_c = percore[c]
        r_t = ranks[c][t_c]
        o = np.argsort(r_t, kind="stable")
        s_e, r_e = grow[s_c[o]], r_t[o]
        first = np.searchsorted(r_e, r_e, side="left")
        k = np.arange(len(r_e)) - first
        slot, row = r_e // 128, r_e % 128
        ridx2 = (base2[slot] + k) * 128 + row
        g2 = np.full(NCH2 * 128, -1, np.int64)
        g2[ridx2] = s_e
        ridx3 = ((base3[slot] + k // 4) * 128 + row) * 4 + (k % 4)
        g3 = np.full(NCH3 * 128 * 4, -1, np.int64)
        g3[ridx3] = s_e
        l2rows.append(g2)
        l3rows.append(g3)

    return dict(
        K2=K2, K3=K3, NCH2=NCH2, NCH3=NCH3,
        orders=orders, l2rows=l2rows, l3rows=l3rows,
    )


def _pack_pm(a, nch):
    """[NCH*128, W] row-major -> [128, nch, W] partition-major contiguous."""
    W = a.shape[1]
    return np.ascontiguousarray(a.reshape(nch, 128, W).transpose(1, 0, 2))


def _expand_l2(c, t0_all, prep):
    NCH = prep["NCH2"]
    g2 = prep["l2rows"][c]
    pad = g2 < 0
    R = t0_all[np.where(pad, 0, g2)]  # [EP, 264] f16
    P = np.empty((NCH * 128, RW), t0_all.dtype)
    P[:, 0:256] = R[:, 0:256]
    P[:, 256:260] = 1.0  # w-slots: weighting writes denominator terms here
    L = R[:, 256:260].copy()
    L[pad] = PAD_LOGIT
    return dict(P=_pack_pm(P, NCH), LAS=_pack_pm(L, NCH))


def _expand_l3(c, t1_all, prep):
    NCH = prep["NCH3"]
    g3 = prep["l3rows"][c]
    pad = (g3 < 0).reshape(-1, 4)
    R = t1_all[np.where(g3 < 0, 0, g3)]  # [EP*4, 66] f16
    EP = NCH * 128
    P = np.empty((EP, RW), t1_all.dtype)
    pv = P.reshape(EP, 65, 4)
    pv[:, 0:64, :] = R[:, 0:64].reshape(EP, 4, 64).transpose(0, 2, 1)
    pv[:, 64, :] = 1.0
    L = R[:, 64].reshape(EP, 4).copy()
    L[pad] = PAD_LOGIT
    return dict(P=_pack_pm(P, NCH), LAS=_pack_pm(L, NCH))


_cache = {}
LAST_PROFILE = {}


def _run(nc, in_maps, core_ids, label):
    trace = bool(int(os.environ.get("GAT_PROFILE", "0")))
    if trace:
        try:
            import sys

            import profile_hook

            profile_hook.install()
            import concourse.bass_utils as bu

            bu.upload_artifacts = lambda tmpdir: "local://skipped"
            tdir = f"/tmp/gat_trace_{label}"
            os.makedirs(tdir, exist_ok=True)
            for f in os.listdir(tdir):
                os.unlink(os.path.join(tdir, f))
            br = run_bass_kernel_spmd(nc, in_maps, core_ids, trace=True, tmpdir=tdir)
            LAST_PROFILE[label] = br.exec_time_ns
            return br.results
        except Exception as e:  # fall back to untraced
            print(f"traced run failed ({e!r}); untraced retry", file=sys.stderr)
    br = run_bass_kernel_spmd(nc, in_maps, core_ids)
    LAST_PROFILE[label] = br.exec_time_ns
    return br.results


def kernel(x, edge_index, W0, att_src0, att_dst0, b0, W1, att_src1, att_dst1, b1):
    x = np.asarray(x, np.float32)
    edge_index = np.asarray(edge_index)
    d = _dims()
    N, NLOC, NP, NT = d["N"], d["NLOC"], d["NP"], d["NT"]

    prep = _prep_edges(edge_index, d)
    key = (prep["K2"], prep["K3"])
    if key not in _cache:
        _cache[key] = (build_l1(d), build_l2(d, prep["K2"]), build_l3(d, prep["K3"]))
    nc1, nc2, nc3 = _cache[key]

    # interleave W0 columns: col e*4+h <- W0 col h*64+e; alphas cols 256..263
    W0f = np.asarray(W0, np.float32)
    W0i = np.ascontiguousarray(
        W0f.reshape(256, 4, 64).transpose(0, 2, 1).reshape(256, 256)
    )
    as0 = W0f.reshape(256, 4, 64) * np.asarray(att_src0, np.float32)[None, :, :]
    ad0 = W0f.reshape(256, 4, 64) * np.asarray(att_dst0, np.float32)[None, :, :]
    W0e = _f16(
        np.concatenate([W0i, as0.sum(axis=2), ad0.sum(axis=2)], axis=1)
    )  # [256, 264]

    # W1e rows permuted to the interleaved feature order
    W1f = np.asarray(W1, np.float32)
    was1 = W1f @ np.asarray(att_src1, np.float32).ravel()
    wad1 = W1f @ np.asarray(att_dst1, np.float32).ravel()
    W1e = np.concatenate([W1f, was1[:, None], wad1[:, None]], axis=1)  # [256, 66]
    perm = (np.arange(256) % 4) * 64 + np.arange(256) // 4

    b0i = np.asarray(b0, np.float32)[(np.arange(256) % 4) * 64 + np.arange(256) // 4]
    B0 = _f16(np.tile(b0i[None, :], (128, 1)))
    W1e16 = _f16(W1e[perm])
    B66 = -(W1e16.astype(np.float32).sum(axis=0))[:, None]  # elu+1 shift corr
    B1 = np.tile(np.asarray(b1, np.float32)[None, :], (128, 1))
    IDN = _f16(np.eye(128, dtype=np.float32))
    core_ids = list(range(NCORES))

    # launch 1
    xb = _f16(x)
    in1 = []
    for c in range(NCORES):
        xT = np.zeros((d["F_IN"], NP), xb.dtype)
        xT[:, :NLOC] = xb[c * NLOC : (c + 1) * NLOC][prep["orders"][c]].T
        in1.append(dict(xT=xT, W0e=W0e))
    r1 = _run(nc1, in1, core_ids, "l1")

    t0_all = np.concatenate([r1[c]["t0"] for c in range(NCORES)], axis=0)

    in2 = []
    for c in range(NCORES):
        e = _expand_l2(c, t0_all, prep)
        t0c = t0_all[c * NP : (c + 1) * NP]
        AD = np.ascontiguousarray(
            t0c[:, 260:264].reshape(NT, 128, 4).transpose(1, 0, 2)
        )
        in2.append(dict(e, AD=AD, IDN=IDN, W1e=W1e16, B0=B0, B66=B66))
    r2 = _run(nc2, in2, core_ids, "l2")

    t1_all = np.concatenate(
        [np.ascontiguousarray(r2[c]["t1T"].T) for c in range(NCORES)], axis=0
    )  # [8*NP, 66] f16

    in3 = []
    for c in range(NCORES):
        e = _expand_l3(c, t1_all, prep)
        t1c = t1_all[c * NP : (c + 1) * NP]
        t65 = t1c[:, 65].copy()
        t65[NLOC:] = 0  # pad ranks carry NaN; zero so 0*NaN can't cross rows
        AD = np.ascontiguousarray(
            np.repeat(t65.reshape(NT, 128).transpose(1, 0)[:, :, None], 4, axis=2)
        )
        in3.append(dict(e, AD=AD, IDN=IDN, B1=B1))
    r3 = _run(nc3, in3, core_ids, "l3")

    out = np.zeros((N, 64), np.float32)
    for c in range(NCORES):
        out[c * NLOC + prep["orders"][c]] = r3[c]["out"][:NLOC]
    return out


# revision 22
# speedup vs baseline: 1.5282x; 1.1665x over previous
"""Two-layer GAT (PyG-style GATConv x2) on 8 Trainium2 NeuronCores.

Design (v3, "rank-identity"): nodes are sharded across the 8 cores by
destination and, per core, PERMUTED BY DEGREE (rank order). Edge rows are
laid out so that chunk k of slot s holds the k-th edge of each of the 128
nodes ranked [128s, 128s+128) -- the segment-sum's placement matrix is then
the IDENTITY for every chunk (loaded from SBUF, never streamed from HBM),
and the softmax denominators ride in 4 w-slot columns. The degree sort
makes per-tile chunk counts nearly uniform so padding stays ~2%.

Payload rows use an INTERLEAVED head layout (col = e*4 + h) so the
per-edge attention weighting runs in the DVE's 2x perf mode (stride-1
last dim; measured 2x vs the blocked layout).

All model arithmetic (matmuls, logits, exp, softmax division, weighting,
ELU, bias) runs on device; the host only gathers/permutes/pads rows and
converts dtypes between the three SPMD launches:
  1. t0[rank, :] = [x@W0 (interleaved) | as | ad] per core (node-major)
  2. layer-0 edge pass -> ELU -> fused per-slot transpose + @W1e
     -> t1T = [feats | as1 | ad1] (rank-major columns)
  3. layer-1 edge pass (quad-packed rows, 4 same-dst edges interleaved)
     -> bias -> output shard (rank-major rows)
"""

import os

import numpy as np

import concourse.bacc as bacc
import concourse.mybir as mybir
from concourse import tile
from concourse.bass_utils import run_bass_kernel_spmd

fp32 = mybir.dt.float32
f16 = mybir.dt.float16
Alu = mybir.AluOpType
Act = mybir.ActivationFunctionType

NCORES = 8
NEG_SLOPE = 0.2
EPS = 1e-16
PAD_LOGIT = -30000.0
CPC = 16  # chunks per payload DMA call
RW = 260  # row width: 65 groups x 4 lanes (64 feat groups + w-slot group)


def _dims():
    return dict(
        N=50000,
        NLOC=6250,
        NP=6272,  # padded to mult of 128
        NT=49,
        F_IN=256,
        HID=256,
        H=4,
        DH=64,
        C_OUT=64,
    )


# ---------------------------------------------------------------- launch 1


def build_l1(d):
    """t0[rank, :] = [x@W0e] node-major per core; W0e = [W0-interleaved |
    W0@A0] folds the per-node attention alphas into the same matmul.
    Stationary = x^T tiles (rank-ordered columns), moving = W0e."""
    nc = bacc.Bacc(None, target_bir_lowering=False, debug=False)
    NP, F, NT = d["NP"], d["F_IN"], d["NT"]

    xT = nc.dram_tensor("xT", [F, NP], f16, kind="ExternalInput")
    W0e = nc.dram_tensor("W0e", [F, 264], f16, kind="ExternalInput")
    t0 = nc.dram_tensor("t0", [NP, 264], f16, kind="ExternalOutput")

    with tile.TileContext(nc) as tc:
        with (
            tc.tile_pool(name="const", bufs=1) as cpool,
            tc.tile_pool(name="work", bufs=3) as pool,
            tc.tile_pool(name="psum", bufs=3, space="PSUM") as pp,
        ):
            w_sb = [
                cpool.tile([128, 264], f16, tag=f"w{k}", name=f"w{k}")
                for k in range(2)
            ]
            xt = [
                cpool.tile([128, NP], f16, tag=f"xt{k}", name=f"xt{k}")
                for k in range(2)
            ]
            for k in range(2):
                nc.sync.dma_start(w_sb[k][:], W0e[128 * k : 128 * (k + 1), :])
                nc.scalar.dma_start(xt[k][:], xT[128 * k : 128 * (k + 1), :])

            TB = 4  # tiles batched per output DMA
            for j0 in range(0, NT, TB):
                nb = min(TB, NT - j0)
                ob = pool.tile([128, TB, 264], f16, tag="ob", name="ob")
                for t in range(nb):
                    j = j0 + t
                    c0 = j * 128
                    ps = pp.tile([128, 264], fp32, tag="ps", name="ps", bufs=4)
                    for k in range(2):
                        nc.tensor.matmul(
                            ps[:],
                            xt[k][:, c0 : c0 + 128],
                            w_sb[k][:],
                            start=(k == 0),
                            stop=(k == 1),
                        )
                    if t % 2 == 0:
                        nc.scalar.activation(ob[:, t, :], ps[:], Act.Copy)
                    else:
                        nc.vector.tensor_copy(ob[:, t, :], ps[:])
                dv = t0[j0 * 128 : (j0 + nb) * 128, :].rearrange(
                    "(t p) f -> p t f", p=128
                )
                nc.sync.dma_start(dv, ob[:, :nb, :])
    nc.compile()
    return nc


# ------------------------------------------------------------ edge machinery


def _edge_stream(nc, tc, d, P, LAS, AD, Ks, idn, fin, pp, tail=None):
    """Shared edge pass for both layers.

    Logits: ewb = exp(lrelu(as + ad_slot)), as per edge row, ad per slot.
    Stream: per call, DMA CPC chunks of 260-wide interleaved payload rows,
    weight by ewb (DVE 2x mode), then per chunk accumulate into the slot
    psum via an identity-stationary matmul (placement = row position).
    """
    NT = d["NT"]
    NCH = sum(Ks)
    base = np.concatenate([[0], np.cumsum(Ks)])

    with (
        tc.tile_pool(name="logit", bufs=1) as lpool,
        tc.tile_pool(name="edge", bufs=1) as epool,
    ):
        las = lpool.tile([128, NCH, 4], f16)
        ewb = lpool.tile([128, NCH, 4], f16)
        ad = lpool.tile([128, NT, 4], f16)
        nc.scalar.dma_start(las[:], LAS[:])
        nc.scalar.dma_start(ad[:], AD[:])

        # ---- logits phase: per slot, e = as + ad[s]; lrelu; exp
        for s in range(NT):
            b0, k = base[s], Ks[s]
            if k <= 0:
                continue
            e_s = las[:, b0 : b0 + k, :]
            adb = ad[:, s : s + 1, :].broadcast_to([128, k, 4])
            nc.vector.tensor_tensor(e_s, e_s, adb, op=Alu.add)
            nc.vector.scalar_tensor_tensor(
                e_s, e_s, NEG_SLOPE, e_s, op0=Alu.mult, op1=Alu.max
            )
            nc.scalar.activation(ewb[:, b0 : b0 + k, :], e_s, Act.Exp)

        # ---- edge streaming
        state = dict(ncalls=0, tiles={})

        def emit_call(call):
            c0 = call * CPC
            nch = min(CPC, NCH - c0)
            G = epool.tile([128, CPC, RW], f16, tag="G", name="G", bufs=6)
            q = nc.sync if call % 2 == 0 else nc.scalar
            q.dma_start(G[:, :nch, :], P[:, c0 : c0 + nch, :])
            g4 = G[:, :nch, :].rearrange("p c (e h) -> p c e h", h=4)
            wb = (
                ewb[:, c0 : c0 + nch, :]
                .unsqueeze(2)
                .broadcast_to([128, nch, RW // 4, 4])
            )
            # payload w-slots are 1.0 from the host, so this multiply also
            # writes the per-lane softmax-denominator columns
            nc.vector.tensor_tensor(g4, g4, wb, op=Alu.mult)
            return G

        c = 0
        for s in range(NT):
            ps = pp.tile([128, RW], fp32, tag="ps", name="ps", bufs=4)
            for k in range(Ks[s]):
                call, cin = c // CPC, c % CPC
                if call >= state["ncalls"]:
                    state["tiles"][call] = emit_call(call)
                    state["ncalls"] = call + 1
                    state["tiles"].pop(call - 5, None)
                G = state["tiles"][call]
                nc.tensor.matmul(
                    ps[:],
                    idn[:],
                    G[:, cin, :],
                    start=(k == 0),
                    stop=(k == Ks[s] - 1),
                )
                c += 1
            fin(s, ps)
            # PE-side tail work lags 2 slots so the in-order PE queue never
            # waits on a finalize chain
            if tail is not None and s >= 2:
                tail(s - 2)
        if tail is not None:
            tail(NT - 2)
            tail(NT - 1)


# ---------------------------------------------------------------- launch 2


def build_l2(d, Ks):
    """Layer-0 edge pass (softmax-div + bias + ELU in finalize), fused with
    a per-slot PE-transpose + @W1e tail -> t1T columns (rank-major)."""
    nc = bacc.Bacc(None, target_bir_lowering=False, debug=False)
    NP, NT = d["NP"], d["NT"]
    NCH = sum(Ks)

    P = nc.dram_tensor("P", [128, NCH, RW], f16, kind="ExternalInput")
    LAS = nc.dram_tensor("LAS", [128, NCH, 4], f16, kind="ExternalInput")
    AD = nc.dram_tensor("AD", [128, NT, 4], f16, kind="ExternalInput")
    IDN = nc.dram_tensor("IDN", [128, 128], f16, kind="ExternalInput")
    W1e = nc.dram_tensor("W1e", [256, 66], f16, kind="ExternalInput")
    B0 = nc.dram_tensor("B0", [128, 256], f16, kind="ExternalInput")
    B66 = nc.dram_tensor("B66", [66, 1], fp32, kind="ExternalInput")
    t1T = nc.dram_tensor("t1T", [66, NP], f16, kind="ExternalOutput")

    with tile.TileContext(nc) as tc:
        with (
            tc.tile_pool(name="const", bufs=1) as cpool,
            tc.tile_pool(name="persist", bufs=1) as ipool,
            tc.tile_pool(name="fin", bufs=3) as fpool,
            tc.tile_pool(name="psum", bufs=1, space="PSUM") as pp,
            tc.tile_pool(name="tpsum", bufs=1, space="PSUM") as tp,
        ):
            idn = cpool.tile([128, 128], f16)
            nc.scalar.dma_start(idn[:], IDN[:])
            b0_sb = cpool.tile([128, 256], f16)
            nc.scalar.dma_start(b0_sb[:], B0[:])
            b66_sb = cpool.tile([66, 1], fp32)
            nc.scalar.dma_start(b66_sb[:], B66[:])
            w1_sb = [
                cpool.tile([128, 66], f16, tag=f"w1_{k}", name=f"w1_{k}")
                for k in range(2)
            ]
            for k in range(2):
                nc.scalar.dma_start(w1_sb[k][:], W1e[128 * k : 128 * (k + 1), :])
            H0 = ipool.tile([128, NT, 256], f16)

            def fin0(s, ps):
                # denominators >= exp(-1) for real rows; pad rows produce
                # Inf/NaN and are discarded by the host.
                # ELU via relu/exp only: H0 = relu(u) + exp(-relu(-u))
                # = elu(u) + 1; the +1 shift is corrected by the B66 bias
                # on the t1 copy (t1 is linear in H0).
                pv = ps[:, 0:256].rearrange("p (e h) -> p e h", h=4)
                rec = fpool.tile([128, 4], fp32, tag="rec", name="rec")
                nc.vector.reciprocal(rec[:], ps[:, 256:260])
                xp = fpool.tile([128, 256], f16, tag="xp", name="xp")
                xv = xp[:].rearrange("p (e h) -> p e h", h=4)
                rb = rec[:].unsqueeze(1).broadcast_to([128, 64, 4])
                nc.vector.tensor_tensor(xv, pv, rb, op=Alu.mult)
                z = fpool.tile([128, 256], f16, tag="z", name="z")
                nc.gpsimd.tensor_tensor(z[:], xp[:], b0_sb[:], op=Alu.add)
                ra = fpool.tile([128, 256], f16, tag="ra", name="ra")
                nc.scalar.activation(ra[:], z[:], Act.Relu)
                rn = fpool.tile([128, 256], f16, tag="rn", name="rn")
                nc.scalar.activation(rn[:], z[:], Act.Relu, scale=-1.0)
                ce = fpool.tile([128, 256], f16, tag="ce", name="ce")
                nc.scalar.activation(ce[:], rn[:], Act.Exp, scale=-1.0)
                nc.gpsimd.tensor_tensor(H0[:, s, :], ra[:], ce[:], op=Alu.add)

            def tail0(s):
                # h0'[slot]^T via PE transpose, then @W1e
                p66 = tp.tile([66, 128], fp32, tag="p66", name="p66", bufs=2)
                for kb in range(2):
                    pt = tp.tile([128, 128], f16, tag="pt", name="pt", bufs=2)
                    nc.tensor.transpose(
                        pt[:], H0[:, s, 128 * kb : 128 * (kb + 1)], idn[:]
                    )
                    hT = fpool.tile([128, 128], f16, tag="hT", name="hT")
                    if kb == 0:
                        nc.scalar.activation(hT[:], pt[:], Act.Copy)
                    else:
                        nc.vector.tensor_copy(hT[:], pt[:])
                    nc.tensor.matmul(
                        p66[:],
                        w1_sb[kb][:],
                        hT[:],
                        start=(kb == 0),
                        stop=(kb == 1),
                    )
                t1b = fpool.tile([66, 128], f16, tag="t1b", name="t1b")
                nc.scalar.activation(t1b[:], p66[:], Act.Identity, bias=b66_sb[:], scale=1.0)
                nc.sync.dma_start(t1T[:, 128 * s : 128 * (s + 1)], t1b[:])

            _edge_stream(nc, tc, d, P, LAS, AD, Ks, idn, fin0, pp, tail=tail0)
    nc.compile()
    return nc


# ---------------------------------------------------------------- launch 3


def build_l3(d, Ks):
    """Layer-1 edge pass, quad-packed (4 same-dst edges per row, lane-
    interleaved); finalize = sum lanes, softmax-div, bias."""
    nc = bacc.Bacc(None, target_bir_lowering=False, debug=False)
    NP, NT, C = d["NP"], d["NT"], d["C_OUT"]
    NCH = sum(Ks)

    P = nc.dram_tensor("P", [128, NCH, RW], f16, kind="ExternalInput")
    LAS = nc.dram_tensor("LAS", [128, NCH, 4], f16, kind="ExternalInput")
    AD = nc.dram_tensor("AD", [128, NT, 4], f16, kind="ExternalInput")
    IDN = nc.dram_tensor("IDN", [128, 128], f16, kind="ExternalInput")
    B1 = nc.dram_tensor("B1", [128, C], fp32, kind="ExternalInput")
    out = nc.dram_tensor("out", [NP, C], fp32, kind="ExternalOutput")

    with tile.TileContext(nc) as tc:
        with (
            tc.tile_pool(name="const", bufs=1) as cpool,
            tc.tile_pool(name="fin", bufs=3) as fpool,
            tc.tile_pool(name="psum", bufs=1, space="PSUM") as pp,
        ):
            idn = cpool.tile([128, 128], f16)
            nc.scalar.dma_start(idn[:], IDN[:])
            b1_sb = cpool.tile([128, C], fp32)
            nc.scalar.dma_start(b1_sb[:], B1[:])

            GB = 4  # slots per batched finalize
            stage = dict(tile=None, s0=0)

            def fin1(s, ps):
                # stage the slot's psum, then finalize GB slots per batch to
                # amortize per-op overheads (l3 slots are only ~5 chunks)
                g = s % GB
                if g == 0:
                    stage["tile"] = fpool.tile(
                        [128, GB, RW], fp32, tag="sb", name="sb", bufs=2
                    )
                    stage["s0"] = s
                sb = stage["tile"]
                nc.scalar.activation(sb[:, g, :], ps[:], Act.Copy)
                if s != NT - 1 and g != GB - 1:
                    return
                n = g + 1
                s0 = stage["s0"]
                sv = sb[:, :n, :].rearrange("p t (e q) -> p t e q", q=4)
                t2 = fpool.tile([128, GB, 65, 2], fp32, tag="t2", name="t2")
                nc.gpsimd.tensor_tensor(
                    t2[:, :n, :, :], sv[:, :, :, 0:2], sv[:, :, :, 2:4], op=Alu.add
                )
                tot = fpool.tile([128, GB, 65], fp32, tag="tot", name="tot")
                nc.vector.tensor_tensor(
                    tot[:, :n, :], t2[:, :n, :, 0], t2[:, :n, :, 1], op=Alu.add
                )
                rec = fpool.tile([128, GB], fp32, tag="recq", name="recq")
                nc.vector.reciprocal(rec[:, :n], tot[:, :n, 64])
                om = fpool.tile([128, GB, C], fp32, tag="om", name="om")
                rb = rec[:, :n].unsqueeze(2).broadcast_to([128, n, C])
                nc.vector.tensor_tensor(
                    om[:, :n, :], tot[:, :n, 0:64], rb, op=Alu.mult
                )
                O = fpool.tile([128, GB, C], fp32, tag="O", name="O")
                bb = b1_sb[:].unsqueeze(1).broadcast_to([128, n, C])
                nc.gpsimd.tensor_tensor(O[:, :n, :], om[:, :n, :], bb, op=Alu.add)
                dv = out[128 * s0 : 128 * (s0 + n), :].rearrange(
                    "(t p) f -> p t f", p=128
                )
                nc.sync.dma_start(dv, O[:, :n, :])

            _edge_stream(nc, tc, d, P, LAS, AD, Ks, idn, fin1, pp)
    nc.compile()
    return nc


# ------------------------------------------------------------ host plumbing


def _f16(a):
    return np.asarray(a).astype(np.float16)


def _prep_edges(edge_index, d):
    """Rank permutations + identity-placement row indices for both layers."""
    N, NLOC, NP, NT = d["N"], d["NLOC"], d["NP"], d["NT"]
    src = np.concatenate([edge_index[0], np.arange(N, dtype=np.int64)])
    dst = np.concatenate([edge_index[1], np.arange(N, dtype=np.int64)])
    core = dst // NLOC

    orders, ranks = [], []
    deg_t, nq_t = [], []
    percore = []
    for c in range(NCORES):
        m = core == c
        s_c, t_c = src[m], (dst[m] - c * NLOC).astype(np.int64)
        percore.append((s_c, t_c))
        deg = np.bincount(t_c, minlength=NLOC)
        order = np.argsort(-deg, kind="stable")
        rank = np.empty(NLOC, np.int64)
        rank[order] = np.arange(NLOC)
        orders.append(order)
        ranks.append(rank)
        dp = np.zeros(NP, np.int64)
        dp[:NLOC] = deg[order]
        deg_t.append(dp.reshape(NT, 128).max(axis=1))
        nqp = np.zeros(NP, np.int64)
        nqp[:NLOC] = (deg[order] + 3) // 4
        nq_t.append(nqp.reshape(NT, 128).max(axis=1))

    K2 = tuple(int(v) for v in np.max(deg_t, axis=0))
    K3 = tuple(int(v) for v in np.max(nq_t, axis=0))
    base2 = np.concatenate([[0], np.cumsum(K2)])
    base3 = np.concatenate([[0], np.cumsum(K3)])
    NCH2, NCH3 = int(base2[-1]), int(base3[-1])

    grow = np.empty(N, np.int64)  # node id -> global table row
    for c in range(NCORES):
        grow[c * NLOC : (c + 1) * NLOC] = c * NP + ranks[c]

    l2rows, l3rows = [], []
    for c in range(NCORES):
        s_c, t_c = percore[c]
        r_t = ranks[c][t_c]
        o = np.argsort(r_t, kind="stable")
        s_e, r_e = grow[s_c[o]], r_t[o]
        first = np.searchsorted(r_e, r_e, side="left")
        k = np.arange(len(r_e)) - first
        slot, row = r_e // 128, r_e % 128
        ridx2 = (base2[slot] + k) * 128 + row
        g2 = np.full(NCH2 * 128, -1, np.int64)
        g2[ridx2] = s_e
        ridx3 = ((base3[slot] + k // 4) * 128 + row) * 4 + (k % 4)
        g3 = np.full(NCH3 * 128 * 4, -1, np.int64)
        g3[ridx3] = s_e
        l2rows.append(g2)
        l3rows.append(g3)

    return dict(
        K2=K2, K3=K3, NCH2=NCH2, NCH3=NCH3,
        orders=orders, l2rows=l2rows, l3rows=l3rows,
    )


def _pack_pm(a, nch):
    """[NCH*128, W] row-major -> [128, nch, W] partition-major contiguous."""
    W = a.shape[1]
    return np.ascontiguousarray(a.reshape(nch, 128, W).transpose(1, 0, 2))


def _expand_l2(c, t0_all, prep):
    NCH = prep["NCH2"]
    g2 = prep["l2rows"][c]
    pad = g2 < 0
    R = t0_all[np.where(pad, 0, g2)]  # [EP, 264] f16
    P = np.empty((NCH * 128, RW), t0_all.dtype)
    P[:, 0:256] = R[:, 0:256]
    P[:, 256:260] = 1.0  # w-slots: weighting writes denominator terms here
    L = R[:, 256:260].copy()
    L[pad] = PAD_LOGIT
    return dict(P=_pack_pm(P, NCH), LAS=_pack_pm(L, NCH))


def _expand_l3(c, t1_all, prep):
    NCH = prep["NCH3"]
    g3 = prep["l3rows"][c]
    pad = (g3 < 0).reshape(-1, 4)
    R = t1_all[np.where(g3 < 0, 0, g3)]  # [EP*4, 66] f16
    EP = NCH * 128
    P = np.empty((EP, RW), t1_all.dtype)
    pv = P.reshape(EP, 65, 4)
    pv[:, 0:64, :] = R[:, 0:64].reshape(EP, 4, 64).transpose(0, 2, 1)
    pv[:, 64, :] = 1.0
    L = R[:, 64].reshape(EP, 4).copy()
    L[pad] = PAD_LOGIT
    return dict(P=_pack_pm(P, NCH), LAS=_pack_pm(L, NCH))


_cache = {}
LAST_PROFILE = {}


def _run(nc, in_maps, core_ids, label):
    trace = bool(int(os.environ.get("GAT_PROFILE", "0")))
    if trace:
        try:
            import sys

            import profile_hook

            profile_hook.install()
            import concourse.bass_utils as bu

            bu.upload_artifacts = lambda tmpdir: "local://skipped"
            tdir = f"/tmp/gat_trace_{label}"
            os.makedirs(tdir, exist_ok=True)
            for f in os.listdir(tdir):
                os.unlink(os.path.join(tdir, f))
            br = run_bass_kernel_spmd(nc, in_maps, core_ids, trace=True, tmpdir=tdir)
            LAST_PROFILE[label] = br.exec_time_ns
            return br.results
        except Exception as e:  # fall back to untraced
            print(f"traced run failed ({e!r}); untraced retry", file=sys.stderr)
    br = run_bass_kernel_spmd(nc, in_maps, core_ids)
    LAST_PROFILE[label] = br.exec_time_ns
    return br.results


def kernel(x, edge_index, W0, att_src0, att_dst0, b0, W1, att_src1, att_dst1, b1):
    x = np.asarray(x, np.float32)
    edge_index = np.asarray(edge_index)
    d = _dims()
    N, NLOC, NP, NT = d["N"], d["NLOC"], d["NP"], d["NT"]

    prep = _prep_edges(edge_index, d)
    key = (prep["K2"], prep["K3"])
    if key not in _cache:
        _cache[key] = (build_l1(d), build_l2(d, prep["K2"]), build_l3(d, prep["K3"]))
    nc1, nc2, nc3 = _cache[key]

    # interleave W0 columns: col e*4+h <- W0 col h*64+e; alphas cols 256..263
    W0f = np.asarray(W0, np.float32)
    W0i = np.ascontiguousarray(
        W0f.reshape(256, 4, 64).transpose(0, 2, 1).reshape(256, 256)
    )
    as0 = W0f.reshape(256, 4, 64) * np.asarray(att_src0, np.float32)[None, :, :]
    ad0 = W0f.reshape(256, 4, 64) * np.asarray(att_dst0, np.float32)[None, :, :]
    W0e = _f16(
        np.concatenate([W0i, as0.sum(axis=2), ad0.sum(axis=2)], axis=1)
    )  # [256, 264]

    # W1e rows permuted to the interleaved feature order
    W1f = np.asarray(W1, np.float32)
    was1 = W1f @ np.asarray(att_src1, np.float32).ravel()
    wad1 = W1f @ np.asarray(att_dst1, np.float32).ravel()
    W1e = np.concatenate([W1f, was1[:, None], wad1[:, None]], axis=1)  # [256, 66]
    perm = (np.arange(256) % 4) * 64 + np.arange(256) // 4

    b0i = np.asarray(b0, np.float32)[(np.arange(256) % 4) * 64 + np.arange(256) // 4]
    B0 = _f16(np.tile(b0i[None, :], (128, 1)))
    W1e16 = _f16(W1e[perm])
    B66 = -(W1e16.astype(np.float32).sum(axis=0))[:, None]  # elu+1 shift corr
    B1 = np.tile(np.asarray(b1, np.float32)[None, :], (128, 1))
    IDN = _f16(np.eye(128, dtype=np.float32))
    core_ids = list(range(NCORES))

    # launch 1
    xb = _f16(x)
    in1 = []
    for c in range(NCORES):
        xT = np.zeros((d["F_IN"], NP), xb.dtype)
        xT[:, :NLOC] = xb[c * NLOC : (c + 1) * NLOC][prep["orders"][c]].T
        in1.append(dict(xT=xT, W0e=W0e))
    r1 = _run(nc1, in1, core_ids, "l1")

    t0_all = np.concatenate([r1[c]["t0"] for c in range(NCORES)], axis=0)

    in2 = []
    for c in range(NCORES):
        e = _expand_l2(c, t0_all, prep)
        t0c = t0_all[c * NP : (c + 1) * NP]
        AD = np.ascontiguousarray(
            t0c[:, 260:264].reshape(NT, 128, 4).transpose(1, 0, 2)
        )
        in2.append(dict(e, AD=AD, IDN=IDN, W1e=W1e16, B0=B0, B66=B66))
    r2 = _run(nc2, in2, core_ids, "l2")

    t1_all = np.concatenate(
        [np.ascontiguousarray(r2[c]["t1T"].T) for c in range(NCORES)], axis=0
    )  # [8*NP, 66] f16

    in3 = []
    for c in range(NCORES):
        e = _expand_l3(c, t1_all, prep)
        t1c = t1_all[c * NP : (c + 1) * NP]
        t65 = t1c[:, 65].copy()
        t65[NLOC:] = 0  # pad ranks carry NaN; zero so 0*NaN can't cross rows
        AD = np.ascontiguousarray(
            np.repeat(t65.reshape(NT, 128).transpose(1, 0)[:, :, None], 4, axis=2)
        )
        in3.append(dict(e, AD=AD, IDN=IDN, B1=B1))
    r3 = _run(nc3, in3, core_ids, "l3")

    out = np.zeros((N, 64), np.float32)
    for c in range(NCORES):
        out[c * NLOC + prep["orders"][c]] = r3[c]["out"][:NLOC]
    return out


# revision 23
# speedup vs baseline: 1.5610x; 1.0215x over previous
"""Two-layer GAT (PyG-style GATConv x2) on 8 Trainium2 NeuronCores.

Design (v3, "rank-identity"): nodes are sharded across the 8 cores by
destination and, per core, PERMUTED BY DEGREE (rank order). Edge rows are
laid out so that chunk k of slot s holds the k-th edge of each of the 128
nodes ranked [128s, 128s+128) -- the segment-sum's placement matrix is then
the IDENTITY for every chunk (loaded from SBUF, never streamed from HBM),
and the softmax denominators ride in 4 w-slot columns. The degree sort
makes per-tile chunk counts nearly uniform so padding stays ~2%.

Payload rows use an INTERLEAVED head layout (col = e*4 + h) so the
per-edge attention weighting runs in the DVE's 2x perf mode (stride-1
last dim; measured 2x vs the blocked layout).

All model arithmetic (matmuls, logits, exp, softmax division, weighting,
ELU, bias) runs on device; the host only gathers/permutes/pads rows and
converts dtypes between the three SPMD launches:
  1. t0[rank, :] = [x@W0 (interleaved) | as | ad] per core (node-major)
  2. layer-0 edge pass -> ELU -> fused per-slot transpose + @W1e
     -> t1T = [feats | as1 | ad1] (rank-major columns)
  3. layer-1 edge pass (quad-packed rows, 4 same-dst edges interleaved)
     -> bias -> output shard (rank-major rows)
"""

import os

import numpy as np

import concourse.bacc as bacc
import concourse.mybir as mybir
from concourse import tile
from concourse.bass_utils import run_bass_kernel_spmd

fp32 = mybir.dt.float32
f16 = mybir.dt.float16
Alu = mybir.AluOpType
Act = mybir.ActivationFunctionType

NCORES = 8
NEG_SLOPE = 0.2
EPS = 1e-16
PAD_LOGIT = -30000.0
CPC = 16  # chunks per payload DMA call
RW = 260  # row width: 65 groups x 4 lanes (64 feat groups + w-slot group)


def _dims():
    return dict(
        N=50000,
        NLOC=6250,
        NP=6272,  # padded to mult of 128
        NT=49,
        F_IN=256,
        HID=256,
        H=4,
        DH=64,
        C_OUT=64,
    )


# ---------------------------------------------------------------- launch 1


def build_l1(d):
    """t0[rank, :] = [x@W0e] node-major per core; W0e = [W0-interleaved |
    W0@A0] folds the per-node attention alphas into the same matmul.
    Stationary = x^T tiles (rank-ordered columns), moving = W0e."""
    nc = bacc.Bacc(None, target_bir_lowering=False, debug=False)
    NP, F, NT = d["NP"], d["F_IN"], d["NT"]

    xT = nc.dram_tensor("xT", [F, NP], f16, kind="ExternalInput")
    W0e = nc.dram_tensor("W0e", [F, 264], f16, kind="ExternalInput")
    t0 = nc.dram_tensor("t0", [NP, 264], f16, kind="ExternalOutput")

    with tile.TileContext(nc) as tc:
        with (
            tc.tile_pool(name="const", bufs=1) as cpool,
            tc.tile_pool(name="work", bufs=3) as pool,
            tc.tile_pool(name="psum", bufs=3, space="PSUM") as pp,
        ):
            w_sb = [
                cpool.tile([128, 264], f16, tag=f"w{k}", name=f"w{k}")
                for k in range(2)
            ]
            xt = [
                cpool.tile([128, NP], f16, tag=f"xt{k}", name=f"xt{k}")
                for k in range(2)
            ]
            for k in range(2):
                nc.sync.dma_start(w_sb[k][:], W0e[128 * k : 128 * (k + 1), :])
            HP = NP // 2
            for h in range(2):
                for k in range(2):
                    nc.scalar.dma_start(
                        xt[k][:, h * HP : (h + 1) * HP],
                        xT[128 * k : 128 * (k + 1), h * HP : (h + 1) * HP],
                    )

            TB = 4  # tiles batched per output DMA
            for j0 in range(0, NT, TB):
                nb = min(TB, NT - j0)
                ob = pool.tile([128, TB, 264], f16, tag="ob", name="ob")
                for t in range(nb):
                    j = j0 + t
                    c0 = j * 128
                    ps = pp.tile([128, 264], fp32, tag="ps", name="ps", bufs=4)
                    for k in range(2):
                        nc.tensor.matmul(
                            ps[:],
                            xt[k][:, c0 : c0 + 128],
                            w_sb[k][:],
                            start=(k == 0),
                            stop=(k == 1),
                        )
                    if t % 2 == 0:
                        nc.scalar.activation(ob[:, t, :], ps[:], Act.Copy)
                    else:
                        nc.vector.tensor_copy(ob[:, t, :], ps[:])
                dv = t0[j0 * 128 : (j0 + nb) * 128, :].rearrange(
                    "(t p) f -> p t f", p=128
                )
                nc.sync.dma_start(dv, ob[:, :nb, :])
    nc.compile()
    return nc


# ------------------------------------------------------------ edge machinery


def _edge_stream(nc, tc, d, P, LAS, AD, Ks, idn, fin, pp, tail=None):
    """Shared edge pass for both layers.

    Logits: ewb = exp(lrelu(as + ad_slot)), as per edge row, ad per slot.
    Stream: per call, DMA CPC chunks of 260-wide interleaved payload rows,
    weight by ewb (DVE 2x mode), then per chunk accumulate into the slot
    psum via an identity-stationary matmul (placement = row position).
    """
    NT = d["NT"]
    NCH = sum(Ks)
    base = np.concatenate([[0], np.cumsum(Ks)])

    with (
        tc.tile_pool(name="logit", bufs=1) as lpool,
        tc.tile_pool(name="edge", bufs=1) as epool,
    ):
        las = lpool.tile([128, NCH, 4], f16)
        ewb = lpool.tile([128, NCH, 4], f16)
        ad = lpool.tile([128, NT, 4], f16)
        nc.scalar.dma_start(las[:], LAS[:])
        nc.scalar.dma_start(ad[:], AD[:])

        # ---- logits phase: per slot, e = as + ad[s]; lrelu; exp
        for s in range(NT):
            b0, k = base[s], Ks[s]
            if k <= 0:
                continue
            e_s = las[:, b0 : b0 + k, :]
            adb = ad[:, s : s + 1, :].broadcast_to([128, k, 4])
            nc.vector.tensor_tensor(e_s, e_s, adb, op=Alu.add)
            nc.vector.scalar_tensor_tensor(
                e_s, e_s, NEG_SLOPE, e_s, op0=Alu.mult, op1=Alu.max
            )
            nc.scalar.activation(ewb[:, b0 : b0 + k, :], e_s, Act.Exp)

        # ---- edge streaming
        state = dict(ncalls=0, tiles={})

        def emit_call(call):
            c0 = call * CPC
            nch = min(CPC, NCH - c0)
            G = epool.tile([128, CPC, RW], f16, tag="G", name="G", bufs=8)
            q = nc.sync if call % 2 == 0 else nc.scalar
            q.dma_start(G[:, :nch, :], P[:, c0 : c0 + nch, :])
            g4 = G[:, :nch, :].rearrange("p c (e h) -> p c e h", h=4)
            wb = (
                ewb[:, c0 : c0 + nch, :]
                .unsqueeze(2)
                .broadcast_to([128, nch, RW // 4, 4])
            )
            # payload w-slots are 1.0 from the host, so this multiply also
            # writes the per-lane softmax-denominator columns
            nc.vector.tensor_tensor(g4, g4, wb, op=Alu.mult)
            return G

        c = 0
        for s in range(NT):
            ps = pp.tile([128, RW], fp32, tag="ps", name="ps", bufs=4)
            for k in range(Ks[s]):
                call, cin = c // CPC, c % CPC
                if call >= state["ncalls"]:
                    state["tiles"][call] = emit_call(call)
                    state["ncalls"] = call + 1
                    state["tiles"].pop(call - 7, None)
                G = state["tiles"][call]
                nc.tensor.matmul(
                    ps[:],
                    idn[:],
                    G[:, cin, :],
                    start=(k == 0),
                    stop=(k == Ks[s] - 1),
                )
                c += 1
            fin(s, ps)
            # PE-side tail work lags 2 slots so the in-order PE queue never
            # waits on a finalize chain
            if tail is not None and s >= 2:
                tail(s - 2)
        if tail is not None:
            tail(NT - 2)
            tail(NT - 1)


# ---------------------------------------------------------------- launch 2


def build_l2(d, Ks, zb0):
    """Layer-0 edge pass (softmax-div + bias + ELU in finalize), fused with
    a per-slot PE-transpose + @W1e tail -> t1T columns (rank-major)."""
    nc = bacc.Bacc(None, target_bir_lowering=False, debug=False)
    NP, NT = d["NP"], d["NT"]
    NCH = sum(Ks)

    P = nc.dram_tensor("P", [128, NCH, RW], f16, kind="ExternalInput")
    LAS = nc.dram_tensor("LAS", [128, NCH, 4], f16, kind="ExternalInput")
    AD = nc.dram_tensor("AD", [128, NT, 4], f16, kind="ExternalInput")
    IDN = nc.dram_tensor("IDN", [128, 128], f16, kind="ExternalInput")
    W1e = nc.dram_tensor("W1e", [256, 66], f16, kind="ExternalInput")
    B0 = nc.dram_tensor("B0", [128, 256], f16, kind="ExternalInput")
    B66 = nc.dram_tensor("B66", [66, 1], fp32, kind="ExternalInput")
    t1T = nc.dram_tensor("t1T", [66, NP], f16, kind="ExternalOutput")

    with tile.TileContext(nc) as tc:
        with (
            tc.tile_pool(name="const", bufs=1) as cpool,
            tc.tile_pool(name="persist", bufs=1) as ipool,
            tc.tile_pool(name="fin", bufs=3) as fpool,
            tc.tile_pool(name="psum", bufs=1, space="PSUM") as pp,
            tc.tile_pool(name="tpsum", bufs=1, space="PSUM") as tp,
        ):
            idn = cpool.tile([128, 128], f16)
            nc.scalar.dma_start(idn[:], IDN[:])
            b0_sb = cpool.tile([128, 256], f16)
            nc.scalar.dma_start(b0_sb[:], B0[:])
            b66_sb = cpool.tile([66, 1], fp32)
            nc.scalar.dma_start(b66_sb[:], B66[:])
            w1_sb = [
                cpool.tile([128, 66], f16, tag=f"w1_{k}", name=f"w1_{k}")
                for k in range(2)
            ]
            for k in range(2):
                nc.scalar.dma_start(w1_sb[k][:], W1e[128 * k : 128 * (k + 1), :])
            H0 = ipool.tile([128, NT, 256], f16)

            def fin0(s, ps):
                # denominators >= exp(-1) for real rows; pad rows produce
                # Inf/NaN and are discarded by the host.
                # ELU via relu/exp only: H0 = relu(u) + exp(-relu(-u))
                # = elu(u) + 1; the +1 shift is corrected by the B66 bias
                # on the t1 copy (t1 is linear in H0).
                pv = ps[:, 0:256].rearrange("p (e h) -> p e h", h=4)
                rec = fpool.tile([128, 4], fp32, tag="rec", name="rec")
                nc.vector.reciprocal(rec[:], ps[:, 256:260])
                xp = fpool.tile([128, 256], f16, tag="xp", name="xp")
                xv = xp[:].rearrange("p (e h) -> p e h", h=4)
                rb = rec[:].unsqueeze(1).broadcast_to([128, 64, 4])
                nc.vector.tensor_tensor(xv, pv, rb, op=Alu.mult)
                if zb0:
                    z = xp
                else:
                    z = fpool.tile([128, 256], f16, tag="z", name="z")
                    nc.gpsimd.tensor_tensor(z[:], xp[:], b0_sb[:], op=Alu.add)
                ra = fpool.tile([128, 256], f16, tag="ra", name="ra")
                nc.scalar.activation(ra[:], z[:], Act.Relu)
                rn = fpool.tile([128, 256], f16, tag="rn", name="rn")
                nc.scalar.activation(rn[:], z[:], Act.Relu, scale=-1.0)
                ce = fpool.tile([128, 256], f16, tag="ce", name="ce")
                nc.scalar.activation(ce[:], rn[:], Act.Exp, scale=-1.0)
                nc.gpsimd.tensor_tensor(H0[:, s, :], ra[:], ce[:], op=Alu.add)

            def tail0(s):
                # h0'[slot]^T via PE transpose, then @W1e
                p66 = tp.tile([66, 128], fp32, tag="p66", name="p66", bufs=2)
                for kb in range(2):
                    pt = tp.tile([128, 128], f16, tag="pt", name="pt", bufs=2)
                    nc.tensor.transpose(
                        pt[:], H0[:, s, 128 * kb : 128 * (kb + 1)], idn[:]
                    )
                    hT = fpool.tile([128, 128], f16, tag="hT", name="hT")
                    if kb == 0:
                        nc.scalar.activation(hT[:], pt[:], Act.Copy)
                    else:
                        nc.vector.tensor_copy(hT[:], pt[:])
                    nc.tensor.matmul(
                        p66[:],
                        w1_sb[kb][:],
                        hT[:],
                        start=(kb == 0),
                        stop=(kb == 1),
                    )
                t1b = fpool.tile([66, 128], f16, tag="t1b", name="t1b")
                nc.scalar.activation(t1b[:], p66[:], Act.Identity, bias=b66_sb[:], scale=1.0)
                nc.sync.dma_start(t1T[:, 128 * s : 128 * (s + 1)], t1b[:])

            _edge_stream(nc, tc, d, P, LAS, AD, Ks, idn, fin0, pp, tail=tail0)
    nc.compile()
    return nc


# ---------------------------------------------------------------- launch 3


def build_l3(d, Ks, zb1):
    """Layer-1 edge pass, quad-packed (4 same-dst edges per row, lane-
    interleaved); finalize = sum lanes, softmax-div, bias."""
    nc = bacc.Bacc(None, target_bir_lowering=False, debug=False)
    NP, NT, C = d["NP"], d["NT"], d["C_OUT"]
    NCH = sum(Ks)

    P = nc.dram_tensor("P", [128, NCH, RW], f16, kind="ExternalInput")
    LAS = nc.dram_tensor("LAS", [128, NCH, 4], f16, kind="ExternalInput")
    AD = nc.dram_tensor("AD", [128, NT, 4], f16, kind="ExternalInput")
    IDN = nc.dram_tensor("IDN", [128, 128], f16, kind="ExternalInput")
    B1 = nc.dram_tensor("B1", [128, C], fp32, kind="ExternalInput")
    out = nc.dram_tensor("out", [NP, C], fp32, kind="ExternalOutput")

    with tile.TileContext(nc) as tc:
        with (
            tc.tile_pool(name="const", bufs=1) as cpool,
            tc.tile_pool(name="fin", bufs=3) as fpool,
            tc.tile_pool(name="psum", bufs=1, space="PSUM") as pp,
        ):
            idn = cpool.tile([128, 128], f16)
            nc.scalar.dma_start(idn[:], IDN[:])
            b1_sb = cpool.tile([128, C], fp32)
            nc.scalar.dma_start(b1_sb[:], B1[:])

            GB = 4  # slots per batched finalize
            stage = dict(tile=None, s0=0)

            def fin1(s, ps):
                # stage the slot's psum, then finalize GB slots per batch to
                # amortize per-op overheads (l3 slots are only ~5 chunks)
                g = s % GB
                if g == 0:
                    stage["tile"] = fpool.tile(
                        [128, GB, RW], fp32, tag="sb", name="sb", bufs=2
                    )
                    stage["s0"] = s
                sb = stage["tile"]
                nc.scalar.activation(sb[:, g, :], ps[:], Act.Copy)
                if s != NT - 1 and g != GB - 1:
                    return
                n = g + 1
                s0 = stage["s0"]
                sv = sb[:, :n, :].rearrange("p t (e q) -> p t e q", q=4)
                t2 = fpool.tile([128, GB, 65, 2], fp32, tag="t2", name="t2")
                nc.gpsimd.tensor_tensor(
                    t2[:, :n, :, :], sv[:, :, :, 0:2], sv[:, :, :, 2:4], op=Alu.add
                )
                tot = fpool.tile([128, GB, 65], fp32, tag="tot", name="tot")
                nc.vector.tensor_tensor(
                    tot[:, :n, :], t2[:, :n, :, 0], t2[:, :n, :, 1], op=Alu.add
                )
                rec = fpool.tile([128, GB], fp32, tag="recq", name="recq")
                nc.vector.reciprocal(rec[:, :n], tot[:, :n, 64])
                om = fpool.tile([128, GB, C], fp32, tag="om", name="om")
                rb = rec[:, :n].unsqueeze(2).broadcast_to([128, n, C])
                nc.vector.tensor_tensor(
                    om[:, :n, :], tot[:, :n, 0:64], rb, op=Alu.mult
                )
                if zb1:
                    O = om
                else:
                    O = fpool.tile([128, GB, C], fp32, tag="O", name="O")
                    bb = b1_sb[:].unsqueeze(1).broadcast_to([128, n, C])
                    nc.gpsimd.tensor_tensor(
                        O[:, :n, :], om[:, :n, :], bb, op=Alu.add
                    )
                dv = out[128 * s0 : 128 * (s0 + n), :].rearrange(
                    "(t p) f -> p t f", p=128
                )
                nc.sync.dma_start(dv, O[:, :n, :])

            _edge_stream(nc, tc, d, P, LAS, AD, Ks, idn, fin1, pp)
    nc.compile()
    return nc


# ------------------------------------------------------------ host plumbing


def _f16(a):
    return np.asarray(a).astype(np.float16)


def _prep_edges(edge_index, d):
    """Rank permutations + identity-placement row indices for both layers."""
    N, NLOC, NP, NT = d["N"], d["NLOC"], d["NP"], d["NT"]
    src = np.concatenate([edge_index[0], np.arange(N, dtype=np.int64)])
    dst = np.concatenate([edge_index[1], np.arange(N, dtype=np.int64)])
    core = dst // NLOC

    orders, ranks = [], []
    deg_t, nq_t = [], []
    percore = []
    for c in range(NCORES):
        m = core == c
        s_c, t_c = src[m], (dst[m] - c * NLOC).astype(np.int64)
        percore.append((s_c, t_c))
        deg = np.bincount(t_c, minlength=NLOC)
        order = np.argsort(-deg, kind="stable")
        rank = np.empty(NLOC, np.int64)
        rank[order] = np.arange(NLOC)
        orders.append(order)
        ranks.append(rank)
        dp = np.zeros(NP, np.int64)
        dp[:NLOC] = deg[order]
        deg_t.append(dp.reshape(NT, 128).max(axis=1))
        nqp = np.zeros(NP, np.int64)
        nqp[:NLOC] = (deg[order] + 3) // 4
        nq_t.append(nqp.reshape(NT, 128).max(axis=1))

    K2 = tuple(int(v) for v in np.max(deg_t, axis=0))
    K3 = tuple(int(v) for v in np.max(nq_t, axis=0))
    base2 = np.concatenate([[0], np.cumsum(K2)])
    base3 = np.concatenate([[0], np.cumsum(K3)])
    NCH2, NCH3 = int(base2[-1]), int(base3[-1])

    grow = np.empty(N, np.int64)  # node id -> global table row
    for c in range(NCORES):
        grow[c * NLOC : (c + 1) * NLOC] = c * NP + ranks[c]

    l2rows, l3rows = [], []
    for c in range(NCORES):
        s_c, t_c = percore[c]
        r_t = ranks[c][t_c]
        o = np.argsort(r_t, kind="stable")
        s_e, r_e = grow[s_c[o]], r_t[o]
        first = np.searchsorted(r_e, r_e, side="left")
        k = np.arange(len(r_e)) - first
        slot, row = r_e // 128, r_e % 128
        ridx2 = (base2[slot] + k) * 128 + row
        g2 = np.full(NCH2 * 128, -1, np.int64)
        g2[ridx2] = s_e
        ridx3 = ((base3[slot] + k // 4) * 128 + row) * 4 + (k % 4)
        g3 = np.full(NCH3 * 128 * 4, -1, np.int64)
        g3[ridx3] = s_e
        l2rows.append(g2)
        l3rows.append(g3)

    return dict(
        K2=K2, K3=K3, NCH2=NCH2, NCH3=NCH3,
        orders=orders, l2rows=l2rows, l3rows=l3rows,
    )


def _pack_pm(a, nch):
    """[NCH*128, W] row-major -> [128, nch, W] partition-major contiguous."""
    W = a.shape[1]
    return np.ascontiguousarray(a.reshape(nch, 128, W).transpose(1, 0, 2))


def _expand_l2(c, t0_all, prep):
    NCH = prep["NCH2"]
    g2 = prep["l2rows"][c]
    pad = g2 < 0
    R = t0_all[np.where(pad, 0, g2)]  # [EP, 264] f16
    P = np.empty((NCH * 128, RW), t0_all.dtype)
    P[:, 0:256] = R[:, 0:256]
    P[:, 256:260] = 1.0  # w-slots: weighting writes denominator terms here
    L = R[:, 256:260].copy()
    L[pad] = PAD_LOGIT
    return dict(P=_pack_pm(P, NCH), LAS=_pack_pm(L, NCH))


def _expand_l3(c, t1_all, prep):
    NCH = prep["NCH3"]
    g3 = prep["l3rows"][c]
    pad = (g3 < 0).reshape(-1, 4)
    R = t1_all[np.where(g3 < 0, 0, g3)]  # [EP*4, 66] f16
    EP = NCH * 128
    P = np.empty((EP, RW), t1_all.dtype)
    pv = P.reshape(EP, 65, 4)
    pv[:, 0:64, :] = R[:, 0:64].reshape(EP, 4, 64).transpose(0, 2, 1)
    pv[:, 64, :] = 1.0
    L = R[:, 64].reshape(EP, 4).copy()
    L[pad] = PAD_LOGIT
    return dict(P=_pack_pm(P, NCH), LAS=_pack_pm(L, NCH))


_cache = {}
LAST_PROFILE = {}


def _run(nc, in_maps, core_ids, label):
    trace = bool(int(os.environ.get("GAT_PROFILE", "0")))
    if trace:
        try:
            import sys

            import profile_hook

            profile_hook.install()
            import concourse.bass_utils as bu

            bu.upload_artifacts = lambda tmpdir: "local://skipped"
            tdir = f"/tmp/gat_trace_{label}"
            os.makedirs(tdir, exist_ok=True)
            for f in os.listdir(tdir):
                os.unlink(os.path.join(tdir, f))
            br = run_bass_kernel_spmd(nc, in_maps, core_ids, trace=True, tmpdir=tdir)
            LAST_PROFILE[label] = br.exec_time_ns
            return br.results
        except Exception as e:  # fall back to untraced
            print(f"traced run failed ({e!r}); untraced retry", file=sys.stderr)
    br = run_bass_kernel_spmd(nc, in_maps, core_ids)
    LAST_PROFILE[label] = br.exec_time_ns
    return br.results


def kernel(x, edge_index, W0, att_src0, att_dst0, b0, W1, att_src1, att_dst1, b1):
    x = np.asarray(x, np.float32)
    edge_index = np.asarray(edge_index)
    d = _dims()
    N, NLOC, NP, NT = d["N"], d["NLOC"], d["NP"], d["NT"]

    prep = _prep_edges(edge_index, d)
    zb0 = not np.any(np.asarray(b0))
    zb1 = not np.any(np.asarray(b1))
    key = (prep["K2"], prep["K3"], zb0, zb1)
    if key not in _cache:
        _cache[key] = (
            build_l1(d),
            build_l2(d, prep["K2"], zb0),
            build_l3(d, prep["K3"], zb1),
        )
    nc1, nc2, nc3 = _cache[key]

    # interleave W0 columns: col e*4+h <- W0 col h*64+e; alphas cols 256..263
    W0f = np.asarray(W0, np.float32)
    W0i = np.ascontiguousarray(
        W0f.reshape(256, 4, 64).transpose(0, 2, 1).reshape(256, 256)
    )
    as0 = W0f.reshape(256, 4, 64) * np.asarray(att_src0, np.float32)[None, :, :]
    ad0 = W0f.reshape(256, 4, 64) * np.asarray(att_dst0, np.float32)[None, :, :]
    W0e = _f16(
        np.concatenate([W0i, as0.sum(axis=2), ad0.sum(axis=2)], axis=1)
    )  # [256, 264]

    # W1e rows permuted to the interleaved feature order
    W1f = np.asarray(W1, np.float32)
    was1 = W1f @ np.asarray(att_src1, np.float32).ravel()
    wad1 = W1f @ np.asarray(att_dst1, np.float32).ravel()
    W1e = np.concatenate([W1f, was1[:, None], wad1[:, None]], axis=1)  # [256, 66]
    perm = (np.arange(256) % 4) * 64 + np.arange(256) // 4

    b0i = np.asarray(b0, np.float32)[(np.arange(256) % 4) * 64 + np.arange(256) // 4]
    B0 = _f16(np.tile(b0i[None, :], (128, 1)))
    W1e16 = _f16(W1e[perm])
    B66 = -(W1e16.astype(np.float32).sum(axis=0))[:, None]  # elu+1 shift corr
    B1 = np.tile(np.asarray(b1, np.float32)[None, :], (128, 1))
    IDN = _f16(np.eye(128, dtype=np.float32))
    core_ids = list(range(NCORES))

    # launch 1
    xb = _f16(x)
    in1 = []
    for c in range(NCORES):
        xT = np.zeros((d["F_IN"], NP), xb.dtype)
        xT[:, :NLOC] = xb[c * NLOC : (c + 1) * NLOC][prep["orders"][c]].T
        in1.append(dict(xT=xT, W0e=W0e))
    r1 = _run(nc1, in1, core_ids, "l1")

    t0_all = np.concatenate([r1[c]["t0"] for c in range(NCORES)], axis=0)

    in2 = []
    for c in range(NCORES):
        e = _expand_l2(c, t0_all, prep)
        t0c = t0_all[c * NP : (c + 1) * NP]
        AD = np.ascontiguousarray(
            t0c[:, 260:264].reshape(NT, 128, 4).transpose(1, 0, 2)
        )
        in2.append(dict(e, AD=AD, IDN=IDN, W1e=W1e16, B0=B0, B66=B66))
    r2 = _run(nc2, in2, core_ids, "l2")

    t1_all = np.concatenate(
        [np.ascontiguousarray(r2[c]["t1T"].T) for c in range(NCORES)], axis=0
    )  # [8*NP, 66] f16

    in3 = []
    for c in range(NCORES):
        e = _expand_l3(c, t1_all, prep)
        t1c = t1_all[c * NP : (c + 1) * NP]
        t65 = t1c[:, 65].copy()
        t65[NLOC:] = 0  # pad ranks carry NaN; zero so 0*NaN can't cross rows
        AD = np.ascontiguousarray(
            np.repeat(t65.reshape(NT, 128).transpose(1, 0)[:, :, None], 4, axis=2)
        )
        in3.append(dict(e, AD=AD, IDN=IDN, B1=B1))
    r3 = _run(nc3, in3, core_ids, "l3")

    out = np.zeros((N, 64), np.float32)
    for c in range(NCORES):
        out[c * NLOC + prep["orders"][c]] = r3[c]["out"][:NLOC]
    return out
